# revision 26
# baseline (speedup 1.0000x reference)
"""MixFFN MoE-routing kernel for Trainium2 (8 NeuronCores, token-parallel).

Math (per token block):
    logits = x @ gate_w.T ; probs = softmax(logits); top2 -> ew [N, E] (dense, rows sum to 1)
    CW1 = x @ W1.T ; CW3 = x @ W3.T
    per expert e:
        w1_e = CW1 + (x @ A1e.T) @ B1e.T
        w3_e = CW3 + (x @ A3e.T) @ B3e.T
        q_e  = silu(w1_e) * (w3_e * ew_e)        (ew commutes into the product)
    out = (sum_e q_e) @ W2.T + sum_e B2e @ (A2e-contraction of q_e)

v3 design (vs v1's identity-broadcast structure; HW exec ~950us vs
v1's ~1.22ms):
  * per-expert w1/w3 live in 4 PSUM banks via DIFF-CHAIN accumulation:
    even banks get CW directly from the big GEMM; odd banks get CW via
    an ACT PSUM->PSUM copy (their has_written bits are seeded once in
    phase 0 and no start=True ever lands on them, so diff packs
    accumulate onto the copied values); each expert-pair pack
    accumulates [B_e ; -B_{e-2}] @ [t_e ; t_{e-2}] (32-row contraction,
    4-way row-group packed) so the previous expert's delta cancels
    exactly.  All 16 identity matmuls of v1 are gone.
  * NO gpsimd tensor ops: concurrent gpsimd TT work starves the DVE's
    SBUF ports and slows vector bf16 ops ~3x (measured); vector PSUM-src
    ops and ACT are immune.  All elementwise runs on vector + ACT.
  * pair-width bf16 DVE ops ([P, 2*NTOK]) amortize the DVE fixed
    overhead; h is a pairwise add tree over the pair tiles.
  * 3 of 8 w3 PSUM-exits are ACT copies (+vector bf16 ew-mult) to
    offload the vector engine, which paces the expert section.
  * software-pipelined emission: iteration i interleaves CW GEMMs of
    tile i with the expert-section packs of tile i-1 and the U-packs of
    tile i-2, so the PE never head-of-line blocks on DVE progress.
  * phase 0: HAM prewarm (dummy matmuls while input DMAs land), bf16
    logits (no fp32 x input at all), lora-down chains overlap the
    softmax, PE-transposes of ew at chain end; identity comes in via
    DMA (make_identity's gpsimd dependency stalls startup).  Dep-free
    junk matmuls pad the main->output transition so the HAM clock gate
    never re-throttles.

Sharding: token-parallel.  Each of the 8 cores gets N/8 = 512 tokens and a
replicated copy of all weights; outputs are disjoint row blocks (no
collectives).  All layout transposes / dtype casts are done host-side.

On-chip layout: feature-on-partition ("transposed"), activations [feat, tok].
"""

import numpy as np

# problem dims (hardcoded per harness contract)
N, D, DFF, E, KTOP, R = 4096, 2048, 8192, 8, 2, 16
NCORES = 8
P = 128

_CACHE = {}


def build_bass(D_=D, DFF_=DFF, E_=E, R_=R, NTOK=N // NCORES, repeat=1):
    """Build the per-core Bass program (same SPMD program on every core)."""
    import concourse.bass as bass
    import concourse.mybir as mybir
    from concourse import bacc
    from concourse.tile import TileContext
    from concourse.masks import make_identity

    dt = mybir.dt
    op = mybir.AluOpType
    AF = mybir.ActivationFunctionType

    KD = D_ // P      # contraction tiles over D
    KF = DFF_ // P    # dff tiles
    MD = D_ // P      # output d tiles
    TT = NTOK // P    # token tiles
    ER = E_ * R_      # stacked expert-rank dim (=128 at full size)
    NCH = 4           # expert-pair packs per tile
    NPRE = 30         # HAM prewarm matmuls

    nc = bacc.Bacc("TRN2", target_bir_lowering=False, debug=False)

    # ---- DRAM I/O ----
    x_bf = nc.dram_tensor("x_bf", [P, KD, NTOK], dt.bfloat16, kind="ExternalInput")
    ident_in = nc.dram_tensor("ident_in", [P, P], dt.float32, kind="ExternalInput")
    gate = nc.dram_tensor("gate", [P, KD, E_], dt.bfloat16, kind="ExternalInput")
    w1t = nc.dram_tensor("w1t", [KF, P, KD, P], dt.bfloat16, kind="ExternalInput")
    w3t = nc.dram_tensor("w3t", [KF, P, KD, P], dt.bfloat16, kind="ExternalInput")
    w2t = nc.dram_tensor("w2t", [MD, P, KF, P], dt.bfloat16, kind="ExternalInput")
    # A pack chains: chain p col slots (16 wide): 0:A1[2p] 16:A1[2p-2]
    # 32:A3[2p] 48:A3[2p-2] 64:A1[2p+1] 80:A1[2p-1] 96:A3[2p+1]
    # 112:A3[2p-1]  (expert indices mod 8)
    amix = nc.dram_tensor("amix", [P, KD, NCH, P], dt.bfloat16, kind="ExternalInput")
    # B diff chains matching amix slots; negative blocks negated host-side,
    # zeroed for p==0
    bmix = nc.dram_tensor("bmix", [KF, P, NCH, P], dt.bfloat16, kind="ExternalInput")
    a2s = nc.dram_tensor("a2s", [KF, P, ER], dt.bfloat16, kind="ExternalInput")
    # B2 stacked to match the U-bank layout: b2s[m, 32j+i, h, n] =
    # B2[4h+j][m*128+n, i] for i<16, zero for i in 16..31
    b2s = nc.dram_tensor("b2s", [MD, P, 2, P], dt.bfloat16, kind="ExternalInput")
    out_t = nc.dram_tensor("out_t", [D_, NTOK], dt.float32, kind="ExternalOutput")

    with TileContext(nc) as tc:
        with (
            tc.tile_pool(name="persist", bufs=1) as persist,
            # stream banks: even (CW via direct GEMM) double-buffered,
            # odd (CW via ident) single-buffered; U accumulators x2
            tc.tile_pool(name="pe1", bufs=2, space="PSUM") as pe1,
            tc.tile_pool(name="pe3", bufs=2, space="PSUM") as pe3,
            tc.tile_pool(name="po1", bufs=1, space="PSUM") as po1,
            tc.tile_pool(name="po3", bufs=1, space="PSUM") as po3,
            tc.tile_pool(name="psum_u", bufs=1, space="PSUM") as ppool_u,
            tc.tile_pool(name="dram", bufs=1, space="DRAM") as dpool,
            tc.tile_pool(name="p0", bufs=4) as p0,
            tc.tile_pool(name="stream", bufs=2) as stream,
        ):
            for _rep in range(repeat):
                # ---------- persistent tiles ----------
                ident_f = persist.tile([P, P], dt.float32)
                nc.sync.dma_start(out=ident_f, in_=ident_in[:, :])
                ident_bf = persist.tile([P, P], dt.bfloat16)
                nc.vector.tensor_copy(ident_bf, ident_f)
                junk_sb = persist.tile([P, NTOK], dt.bfloat16, tag="junk")
                nc.vector.memset(junk_sb, 0.5)

                xbf = persist.tile([P, KD, NTOK], dt.bfloat16)
                nc.sync.dma_start(out=xbf, in_=x_bf[:, :, :])
                gsb = persist.tile([P, KD, E_], dt.bfloat16, tag="gsb")
                nc.sync.dma_start(out=gsb, in_=gate[:, :, :])
                amx = persist.tile([P, KD, NCH, P], dt.bfloat16, tag="amx")
                nc.sync.dma_start(out=amx, in_=amix[:, :, :, :])

                h_big = persist.tile([P, KF, NTOK], dt.bfloat16)
                ewT_sb = persist.tile([E_, NTOK], dt.bfloat16)
                ew_b = []
                for e in range(E_):
                    ewb_t = persist.tile([P, NTOK], dt.bfloat16, tag=f"ewb{e}")
                    ew_b.append(ewb_t)
                tmix = []
                for c in range(NCH):
                    t_t = persist.tile([P, NTOK], dt.bfloat16, tag=f"tmix{c}")
                    tmix.append(t_t)

                # ---------- phase 0 ----------
                # HAM prewarm: junk matmuls while the input DMAs land.
                junk_ps3 = po3.tile([P, NTOK], dt.float32, tag="o3")
                for i in range(3):
                    nc.tensor.matmul(
                        junk_ps3, lhsT=ident_bf, rhs=junk_sb,
                        start=True, stop=True,
                    )
                junk_ps = po1.tile([P, NTOK], dt.float32, tag="o1")
                for i in range(NPRE):
                    nc.tensor.matmul(
                        junk_ps, lhsT=ident_bf, rhs=junk_sb,
                        start=True, stop=True,
                    )

                ew_td = dpool.tile([E_, NTOK], dt.bfloat16)
                # logits (bf16 inputs, fp32 psum) then DVE softmax; the
                # lora-down chains keep the PE busy while softmax runs.
                ewts = []
                for tt in range(TT):
                    lg = pe1.tile([P, E_], dt.float32, tag="e1")
                    for k in range(KD):
                        nc.tensor.matmul(
                            lg,
                            lhsT=xbf[:, k, tt * P:(tt + 1) * P],
                            rhs=gsb[:, k, :],
                            start=(k == 0),
                            stop=(k == KD - 1),
                        )
                    # softmax chain on DVE/ACT for this token tile
                    l_sb = p0.tile([P, E_], dt.float32, tag="lsb")
                    nc.vector.tensor_copy(l_sb, lg)
                    m1 = p0.tile([P, 1], dt.float32, tag="m1")
                    nc.vector.reduce_max(m1, l_sb, axis=mybir.AxisListType.X)
                    nm1 = p0.tile([P, 1], dt.float32, tag="nm1")
                    nc.vector.tensor_scalar_mul(nm1, m1, -1.0)
                    mask1 = p0.tile([P, E_], dt.float32, tag="mask1")
                    nc.vector.tensor_scalar(
                        mask1, l_sb, scalar1=m1, scalar2=None, op0=op.is_equal
                    )
                    l2 = p0.tile([P, E_], dt.float32, tag="l2")
                    nc.vector.scalar_tensor_tensor(
                        l2, in0=mask1, scalar=-1e30, in1=l_sb, op0=op.mult, op1=op.add
                    )
                    m2 = p0.tile([P, 1], dt.float32, tag="m2")
                    nc.vector.reduce_max(m2, l2, axis=mybir.AxisListType.X)
                    mask2 = p0.tile([P, E_], dt.float32, tag="mask2")
                    nc.vector.tensor_scalar(
                        mask2, l2, scalar1=m2, scalar2=None, op0=op.is_equal
                    )
                    mask = p0.tile([P, E_], dt.float32, tag="mask")
                    nc.vector.tensor_tensor(mask, mask1, mask2, op=op.add)
                    pexp = p0.tile([P, E_], dt.float32, tag="pexp")
                    nc.scalar.activation(pexp, l_sb, AF.Exp, bias=nm1, scale=1.0)
                    pm = p0.tile([P, E_], dt.float32, tag="pm")
                    nc.vector.tensor_tensor(pm, pexp, mask, op=op.mult)
                    den = p0.tile([P, 1], dt.float32, tag="den")
                    nc.vector.reduce_sum(den, pm, axis=mybir.AxisListType.X)
                    rec = p0.tile([P, 1], dt.float32, tag="rec")
                    nc.vector.reciprocal(rec, den)
                    ewt = p0.tile([P, E_], dt.float32, tag="ewt")
                    nc.vector.tensor_scalar_mul(ewt, pm, rec)
                    ewts.append(ewt)

                # lora-down chains (PE) overlap the softmax chains (DVE)
                for c in range(NCH):
                    tp = pe3.tile([P, NTOK], dt.float32, tag="e3")
                    for k in range(KD):
                        nc.tensor.matmul(
                            tp,
                            lhsT=amx[:, k, c, :],
                            rhs=xbf[:, k, :],
                            start=(k == 0),
                            stop=(k == KD - 1),
                        )
                    nc.scalar.copy(tmix[c], tp)

                # transpose ew [tok, E] -> [E, tok] on the PE (softmax is
                # done by now), then DRAM roundtrip to broadcast rows.
                for tt in range(TT):
                    ewtp = pe3.tile([E_, P], dt.float32, tag="e3")
                    nc.tensor.transpose(ewtp, ewts[tt], ident_f)
                    nc.scalar.copy(ewT_sb[:, tt * P:(tt + 1) * P], ewtp)
                nc.sync.dma_start(out=ew_td, in_=ewT_sb)
                for e in range(E_):
                    src = bass.AP(
                        tensor=ew_td.tensor,
                        offset=ew_td.offset + e * NTOK,
                        ap=[[0, P], [1, NTOK]],
                    )
                    nc.sync.dma_start(out=ew_b[e], in_=src)

                # ---------- U accumulators ----------
                u_ps_a = ppool_u.tile([P, NTOK], dt.float32, tag="ua")
                u_ps_b = ppool_u.tile([P, NTOK], dt.float32, tag="ub")
                u_ps = [u_ps_a, u_ps_b]
                nc.vector.memset(u_ps_a, 0.0)
                nc.vector.memset(u_ps_b, 0.0)

                # ---------- main loop (software pipelined) ----------
                state = {}

                def u_pack(bank, t):
                    st = state[t]
                    for j in range(4):
                        e = 4 * bank + j
                        qp = st["qp"][2 * bank + j // 2]
                        nc.tensor.matmul(
                            u_ps[bank][32 * j:32 * j + R_, :],
                            lhsT=st["a2"][:, e * R_:(e + 1) * R_],
                            rhs=qp[:, j % 2, :],
                            start=(t == 0),
                            stop=(t == KF - 1),
                            tile_position=(0, 32 * j),
                        )

                def start_tile(t):
                    st = {"qp": {}, "sp": {}, "w3p": {}}
                    state[t] = st
                    w1sl = stream.tile([P, KD, P], dt.bfloat16, tag="w1sl")
                    nc.sync.dma_start(out=w1sl, in_=w1t[t, :, :, :])
                    w3sl = stream.tile([P, KD, P], dt.bfloat16, tag="w3sl")
                    nc.sync.dma_start(out=w3sl, in_=w3t[t, :, :, :])
                    bmk = stream.tile([P, NCH, P], dt.bfloat16, tag="bmk", bufs=2)
                    nc.sync.dma_start(out=bmk, in_=bmix[t, :, :, :])
                    a2kt = stream.tile([P, ER], dt.bfloat16, tag="a2kt", bufs=3)
                    nc.sync.dma_start(out=a2kt, in_=a2s[t, :, :])
                    st["wsl"] = (w1sl, w3sl)
                    st["bmk"] = bmk
                    st["a2"] = a2kt
                    st["banks"] = (
                        pe1.tile([P, NTOK], dt.float32, tag="e1", name="e1b"),
                        pe3.tile([P, NTOK], dt.float32, tag="e3", name="e3b"),
                        po1.tile([P, NTOK], dt.float32, tag="o1", name="o1b"),
                        po3.tile([P, NTOK], dt.float32, tag="o3", name="o3b"),
                    )

                def cw_gemm(t, which, half):
                    st = state[t]
                    tgt = st["banks"][which]
                    src = st["wsl"][which]
                    for k in range(8 * half, 8 * half + 8):
                        nc.tensor.matmul(
                            tgt, lhsT=src[:, k, :], rhs=xbf[:, k, :],
                            start=(k == 0), stop=False,
                            skip_group_check=True,
                        )

                def cw_replicate(t):
                    # ACT copies CW fp32 PSUM->PSUM into the odd banks; the
                    # odd banks' has_written bits were seeded in phase 0 and
                    # never cleared (no start=True ever lands on them), so
                    # the diff packs accumulate onto the copied CW values.
                    st = state[t]
                    nc.scalar.copy(st["banks"][2], st["banks"][0])
                    nc.scalar.copy(st["banks"][3], st["banks"][1])

                def pack(t, p):
                    st = state[t]
                    bmk = st["bmk"]
                    last = (p == NCH - 1)
                    # emit the latest-resolving slot (odd3, gated on vector's
                    # w3q) first so the remaining slots chase it back-to-back
                    # into concurrent row groups
                    for j in (3, 2, 1, 0):
                        tgt = st["banks"][j]
                        r0 = 32 * j
                        nc.tensor.matmul(
                            tgt,
                            lhsT=bmk[r0:r0 + 32, p, :],
                            rhs=tmix[p][r0:r0 + 32, :],
                            start=False, stop=last,
                            tile_position=(r0, 0),
                            skip_group_check=True,
                        )

                # experts 1, 3, 5 get their w3 PSUM-exit via an ACT copy
                # (ACT is immune to engine interference and has slack);
                # the remaining 5 stay as vector PSUM-mults
                ACT_W3 = (1, 3, 6)

                def silu_w3q(t, p):
                    st = state[t]
                    e1b, e3b, o1b, o3b = st["banks"]
                    # pair tiles: bf16 elementwise runs at [P, 2*NTOK] width
                    # so the DVE per-op overhead amortizes 2x while keeping
                    # the FIFO latency quantum small
                    sp = stream.tile(
                        [P, 2, NTOK], dt.bfloat16, tag="sp", bufs=5, name="sp")
                    w3p = stream.tile(
                        [P, 2, NTOK], dt.bfloat16, tag="w3p", bufs=5, name="w3p")
                    st["sp"][p] = sp
                    st["w3p"][p] = w3p
                    # odd expert first: releases the single-buffered odd
                    # banks earliest
                    for ee, w1b, w3b in ((2 * p + 1, o1b, o3b), (2 * p, e1b, e3b)):
                        j = ee % 2
                        nc.scalar.activation(sp[:, j, :], w1b, AF.Silu)
                        if ee in ACT_W3:
                            w3r = stream.tile(
                                [P, NTOK], dt.bfloat16, tag="w3r", bufs=2,
                                name="w3r")
                            nc.scalar.copy(w3r, w3b)
                            nc.vector.tensor_tensor(
                                w3p[:, j, :], w3r, ew_b[ee], op=op.mult)
                        else:
                            nc.vector.tensor_tensor(
                                w3p[:, j, :], w3b, ew_b[ee], op=op.mult)

                def q_pair(t, p):
                    st = state[t]
                    qp = stream.tile(
                        [P, 2, NTOK], dt.bfloat16, tag="qp", bufs=6, name="qp")
                    nc.vector.tensor_tensor(
                        qp, st["sp"][p], st["w3p"][p], op=op.mult)
                    st["qp"][p] = qp

                def h_ops(t, step):
                    st = state[t]
                    if step == 0:
                        v01 = stream.tile(
                            [P, 2, NTOK], dt.bfloat16, tag="v01", bufs=2,
                            name="v01")
                        nc.vector.tensor_tensor(
                            v01, st["qp"][0], st["qp"][1], op=op.add)
                        st["v01"] = v01
                    elif step == 1:
                        v23 = stream.tile(
                            [P, 2, NTOK], dt.bfloat16, tag="v23", bufs=2,
                            name="v23")
                        nc.vector.tensor_tensor(
                            v23, st["qp"][2], st["qp"][3], op=op.add)
                        st["v23"] = v23
                    elif step == 2:
                        vv = stream.tile(
                            [P, 2, NTOK], dt.bfloat16, tag="vv", bufs=2,
                            name="vv")
                        nc.vector.tensor_tensor(
                            vv, st["v01"], st["v23"], op=op.add)
                        st["vv"] = vv
                    else:
                        nc.vector.tensor_tensor(
                            h_big[:, t, :], st["vv"][:, 0, :], st["vv"][:, 1, :],
                            op=op.add)

                def junk_fill(n):
                    # dep-free PE filler across the main->output transition
                    # so the HAM clock gate never sees a >3.4us idle window
                    jt = pe1.tile([P, NTOK], dt.float32, tag="e1", name="jt")
                    for _ in range(n):
                        nc.tensor.matmul(
                            jt, lhsT=ident_bf, rhs=junk_sb,
                            start=True, stop=True,
                        )

                for t in range(KF + 2):
                    a = t - 1   # tile in expert-section stage
                    b = t - 2   # tile in tail/U stage
                    if t < KF:
                        start_tile(t)
                        cw_gemm(t, 0, 0)
                    elif t == KF:
                        junk_fill(3)
                    else:
                        junk_fill(6)
                    if 0 <= a < KF:
                        cw_replicate(a)
                        pack(a, 0)
                        silu_w3q(a, 0)
                    if 0 <= b < KF:
                        q_pair(b, 3)
                        h_ops(b, 1)
                    if t < KF:
                        cw_gemm(t, 0, 1)
                    elif t == KF:
                        junk_fill(4)
                    if 0 <= a < KF:
                        pack(a, 1)
                        silu_w3q(a, 1)
                        q_pair(a, 0)
                    if 0 <= b < KF:
                        h_ops(b, 2)
                        h_ops(b, 3)
                        u_pack(0, b)
                    if t < KF:
                        cw_gemm(t, 1, 0)
                    elif t == KF:
                        junk_fill(4)
                    if 0 <= a < KF:
                        pack(a, 2)
                        silu_w3q(a, 2)
                        q_pair(a, 1)
                        h_ops(a, 0)
                    if t < KF:
                        cw_gemm(t, 1, 1)
                    elif t == KF:
                        junk_fill(4)
                    if 0 <= a < KF:
                        pack(a, 3)
                        silu_w3q(a, 3)
                        q_pair(a, 2)
                    if 0 <= b < KF:
                        u_pack(1, b)
                        del state[b]

                # ---------- export U banks ----------
                uq2 = []
                for bank in range(2):
                    uq_t = persist.tile([P, NTOK], dt.bfloat16, tag=f"uq{bank}")
                    nc.vector.tensor_copy(uq_t, u_ps[bank])
                    uq2.append(uq_t)

                # ---------- output GEMM: out = W2 @ H + B2stack @ Uqall ----------
                # w2m half-slabs are prefetched one step ahead so the PE
                # never waits on the 512KB loads at phase entry.
                KH = KF // 2
                w2q = []
                for h in range(2):
                    w2m = stream.tile(
                        [P, KH, P], dt.bfloat16, tag="w2m", bufs=2, name="w2m")
                    nc.sync.dma_start(out=w2m, in_=w2t[0, :, h * KH:(h + 1) * KH, :])
                    w2q.append(w2m)
                b2q = stream.tile([P, 2, P], dt.bfloat16, tag="b2m", bufs=2, name="b2q")
                nc.sync.dma_start(out=b2q, in_=b2s[0, :, :, :])
                for m in range(MD):
                    outp = pe1.tile([P, NTOK], dt.float32, tag="e1")
                    w2h, b2m = w2q, b2q
                    if m + 1 < MD:
                        w2q = []
                        for h in range(2):
                            w2m = stream.tile(
                                [P, KH, P], dt.bfloat16, tag="w2m", bufs=2,
                                name="w2m")
                            nc.sync.dma_start(
                                out=w2m, in_=w2t[m + 1, :, h * KH:(h + 1) * KH, :])
                            w2q.append(w2m)
                        b2q = stream.tile(
                            [P, 2, P], dt.bfloat16, tag="b2m", bufs=2, name="b2q")
                        nc.sync.dma_start(out=b2q, in_=b2s[m + 1, :, :, :])
                    for h in range(2):
                        for kk in range(KH):
                            kt = h * KH + kk
                            nc.tensor.matmul(
                                outp, lhsT=w2h[h][:, kk, :], rhs=h_big[:, kt, :],
                                start=(kt == 0), stop=False,
                            )
                    nc.tensor.matmul(
                        outp, lhsT=b2m[:, 0, :], rhs=uq2[0], start=False, stop=False,
                    )
                    nc.tensor.matmul(
                        outp, lhsT=b2m[:, 1, :], rhs=uq2[1], start=False, stop=True,
                    )
                    osb = stream.tile([P, NTOK], dt.float32, tag="osb")
                    nc.scalar.copy(osb, outp)
                    nc.sync.dma_start(out=out_t[m * P:(m + 1) * P, :], in_=osb)

    nc.compile()
    return nc


def _sw_d(arr):
    """[D, ...] -> [P, KD, ...] partition-major swizzle (d = k*128 + p)."""
    D_ = arr.shape[0]
    rest = arr.shape[1:]
    return np.ascontiguousarray(
        arr.reshape(D_ // 128, 128, *rest).swapaxes(0, 1)
    )


def _pack_amix(A1, A3):
    """A1/A3 [E, R, D] -> [P, KD, 4, 128] diff chains.

    chain p col slots (16 wide): [A1[2p], A1[2p-2], A3[2p], A3[2p-2],
    A1[2p+1], A1[2p-1], A3[2p+1], A3[2p-1]] (indices mod 8)."""
    E_, R_, D_ = A1.shape
    out = np.zeros((D_, 4, 128), A1.dtype)
    for p in range(4):
        sl = [
            A1[2 * p], A1[(2 * p - 2) % 8], A3[2 * p], A3[(2 * p - 2) % 8],
            A1[2 * p + 1], A1[(2 * p - 1) % 8], A3[2 * p + 1], A3[(2 * p - 1) % 8],
        ]
        for j, A in enumerate(sl):
            out[:, p, 16 * j:16 * j + R_] = A.T
    return _sw_d(out)


def _pack_bmix(B1, B3):
    """B1/B3 [E, F, R] -> [KF, 128, 4, 128] diff chains.

    bmix[kt, r, p, m] rows (16 wide): [+B1[2p], -B1[2p-2], +B3[2p],
    -B3[2p-2], +B1[2p+1], -B1[2p-1], +B3[2p+1], -B3[2p-1]]; the negative
    blocks are zero for p == 0 (each tile's chains restart from CW)."""
    E_, F_, R_ = B1.shape
    out = np.zeros((128, 4, F_), B1.dtype)
    for p in range(4):
        sl = [
            (B1[2 * p], 1.0),
            (B1[(2 * p - 2) % 8], -1.0 if p > 0 else 0.0),
            (B3[2 * p], 1.0),
            (B3[(2 * p - 2) % 8], -1.0 if p > 0 else 0.0),
            (B1[2 * p + 1], 1.0),
            (B1[(2 * p - 1) % 8], -1.0 if p > 0 else 0.0),
            (B3[2 * p + 1], 1.0),
            (B3[(2 * p - 1) % 8], -1.0 if p > 0 else 0.0),
        ]
        for j, (B, sgn) in enumerate(sl):
            if sgn != 0.0:
                out[16 * j:16 * j + R_, p, :] = sgn * B.T
    # [row, p, (kt n)] -> [kt, row, p, n]
    return np.ascontiguousarray(
        out.reshape(128, 4, F_ // 128, 128).transpose(2, 0, 1, 3)
    )


def _pack_b2(B2):
    """B2 [E, D, R] -> [MD, 128, 2, 128] matching the U-bank strip layout.

    out[m, 32j+i, h, n] = B2[4h+j][m*128+n, i] for i<16, zeros at i>=16."""
    E_, D_, R_ = B2.shape
    out = np.zeros((128, 2, D_), B2.dtype)
    for h in range(2):
        for j in range(4):
            out[32 * j:32 * j + R_, h, :] = B2[4 * h + j].T
    # [r, h, (m n)] -> [m, r, h, n]
    return np.ascontiguousarray(
        out.reshape(128, 2, D_ // 128, 128).transpose(2, 0, 1, 3)
    )


def _pack_w_ktiles(WT):
    """WT [K, M] (contraction-major) -> [MT, P, KT, P] where
    out[mt, p, kt, n] = WT[kt*128+p, mt*128+n] — per-(mt) slab is
    partition-major with [KT, 128] contiguous per partition."""
    K_, M_ = WT.shape
    return np.ascontiguousarray(
        WT.reshape(K_ // 128, 128, M_ // 128, 128).transpose(2, 1, 0, 3)
    )


def _prep_inputs(x, W1, W3, W2, gate_w, A1, B1, A3, B3, A2, B2):
    """Host-side packing: transposes + casts, shared across cores."""
    import ml_dtypes

    bf16 = ml_dtypes.bfloat16
    f32 = np.float32

    xT = np.ascontiguousarray(np.asarray(x, f32).T)            # [D, N]
    dff = W1.shape[0]
    shared = {
        "gate": _sw_d(np.ascontiguousarray(np.asarray(gate_w, f32).T)).astype(bf16),
        "w1t": _pack_w_ktiles(np.asarray(W1, f32).T.astype(bf16)),
        "w3t": _pack_w_ktiles(np.asarray(W3, f32).T.astype(bf16)),
        "w2t": _pack_w_ktiles(np.asarray(W2, f32).T.astype(bf16)),
        "amix": _pack_amix(np.asarray(A1, f32), np.asarray(A3, f32)).astype(bf16),
        "bmix": _pack_bmix(np.asarray(B1, f32), np.asarray(B3, f32)).astype(bf16),
        "a2s": np.ascontiguousarray(
            np.asarray(A2, f32).transpose(2, 0, 1).reshape(dff // 128, 128, -1)
        ).astype(bf16),
        "b2s": _pack_b2(np.asarray(B2, f32)).astype(bf16),
    }
    shared["ident_in"] = np.eye(128, dtype=f32)
    ntok = xT.shape[1] // NCORES
    in_maps = []
    for c in range(NCORES):
        sl = np.ascontiguousarray(xT[:, c * ntok:(c + 1) * ntok])
        m = dict(shared)
        m["x_bf"] = _sw_d(sl.astype(bf16))
        in_maps.append(m)
    return in_maps


def _ensure_compiled():
    if "exec" not in _CACHE:
        nc = build_bass()
        _CACHE["nc"] = nc
        _CACHE["exec"] = _make_exec(nc)
    return _CACHE["exec"]


def _make_exec(nc):
    """Build a jitted 8-core shard_map executor for a Bass program.

    Mirrors concourse.bass2jax.run_bass_via_pjrt, but caches the jitted
    callable and keeps real inputs un-donated so device buffers can be
    reused across calls (for timing)."""
    import jax
    import concourse.mybir as mybir
    from concourse import bass2jax
    from jax.experimental.shard_map import shard_map
    from jax.sharding import Mesh, PartitionSpec

    bass2jax.install_neuronx_cc_hook()

    partition_name = (
        nc.partition_id_tensor.name if nc.partition_id_tensor else None
    )
    in_names, out_names, out_avals, zero_outs = [], [], [], []
    for alloc in nc.m.functions[0].allocations:
        if not isinstance(alloc, mybir.MemoryLocationSet):
            continue
        name = alloc.memorylocations[0].name
        if alloc.kind == "ExternalInput":
            if name != partition_name:
                in_names.append(name)
        elif alloc.kind == "ExternalOutput":
            np_dtype = mybir.dt.np(alloc.dtype)
            out_names.append(name)
            out_avals.append(
                jax.core.ShapedArray(tuple(alloc.tensor_shape), np_dtype)
            )
            zero_outs.append(np.zeros(tuple(alloc.tensor_shape), np_dtype))

    n_params = len(in_names)
    n_outs = len(out_names)
    all_names = in_names + out_names
    if partition_name is not None:
        all_names = all_names + [partition_name]

    def _body(*args):
        operands = list(args)
        if partition_name is not None:
            operands.append(bass2jax.partition_id_tensor())
        outs = bass2jax._bass_exec_p.bind(
            *operands,
            out_avals=tuple(out_avals),
            in_names=tuple(all_names),
            out_names=tuple(out_names),
            lowering_input_output_aliases=(),
            sim_require_finite=True,
            sim_require_nnan=True,
            nc=nc,
        )
        return tuple(outs)

    devices = jax.devices()[:NCORES]
    mesh = Mesh(np.asarray(devices), ("core",))
    in_specs = (PartitionSpec("core"),) * (n_params + n_outs)
    out_specs = (PartitionSpec("core"),) * n_outs
    donate = tuple(range(n_params, n_params + n_outs))
    sharded = jax.jit(
        shard_map(
            _body, mesh=mesh, in_specs=in_specs, out_specs=out_specs,
            check_rep=False,
        ),
        donate_argnums=donate,
        keep_unused=True,
    )
    ctx = {
        "fn": sharded,
        "body": _body,
        "n_operands": n_params + n_outs,
        "in_names": in_names,
        "out_names": out_names,
        "zero_outs": zero_outs,
        "mesh": mesh,
    }
    return ctx


def _concat_inputs(in_maps, in_names):
    return [
        np.concatenate([in_maps[c][nm] for c in range(NCORES)], axis=0)
        for nm in in_names
    ]


def _run(ctx, concat_in):
    zeros = [
        np.zeros((NCORES * z.shape[0], *z.shape[1:]), z.dtype)
        for z in ctx["zero_outs"]
    ]
    return ctx["fn"](*concat_in, *zeros)


def kernel(x, W1, W3, W2, gate_w, A1, B1, A3, B3, A2, B2):
    ctx = _ensure_compiled()
    in_maps = _prep_inputs(x, W1, W3, W2, gate_w, A1, B1, A3, B3, A2, B2)
    concat_in = _concat_inputs(in_maps, ctx["in_names"])
    out_arrs = _run(ctx, concat_in)
    ntok = N // NCORES
    res = np.asarray(out_arrs[ctx["out_names"].index("out_t")])
    res = res.reshape(NCORES, D, ntok)
    out = np.empty((N, D), np.float32)
    for c in range(NCORES):
        out[c * ntok:(c + 1) * ntok, :] = res[c].T
    return out


def time_device(inputs, iters=3, ctx=None):
    """Upload all operands once (no donation), then wall-time jitted runs."""
    import time as _time

    import jax
    from jax.experimental.shard_map import shard_map
    from jax.sharding import NamedSharding, PartitionSpec, Mesh

    if ctx is None:
        ctx = _ensure_compiled()
    if "fn_nodonate" not in ctx:
        ctx["fn_nodonate"] = jax.jit(
            shard_map(
                ctx["body"], mesh=ctx["mesh"],
                in_specs=(PartitionSpec("core"),) * ctx["n_operands"],
                out_specs=(PartitionSpec("core"),) * len(ctx["out_names"]),
                check_rep=False,
            ),
            keep_unused=True,
        )
    fn = ctx["fn_nodonate"]
    in_maps = _prep_inputs(**inputs)
    concat_in = _concat_inputs(in_maps, ctx["in_names"])
    zeros = [
        np.zeros((NCORES * z.shape[0], *z.shape[1:]), z.dtype)
        for z in ctx["zero_outs"]
    ]
    sh = NamedSharding(ctx["mesh"], PartitionSpec("core"))
    dev = [jax.device_put(a, sh) for a in (concat_in + zeros)]
    jax.block_until_ready(fn(*dev))  # warmup/compile
    times = []
    for _ in range(iters):
        t0 = _time.perf_counter()
        jax.block_until_ready(fn(*dev))
        times.append(_time.perf_counter() - t0)
    return min(times)


# revision 27
# speedup vs baseline: 1.0017x; 1.0017x over previous
"""MixFFN MoE-routing kernel for Trainium2 (8 NeuronCores, token-parallel).

Math (per token block):
    logits = x @ gate_w.T ; probs = softmax(logits); top2 -> ew [N, E] (dense, rows sum to 1)
    CW1 = x @ W1.T ; CW3 = x @ W3.T
    per expert e:
        w1_e = CW1 + (x @ A1e.T) @ B1e.T
        w3_e = CW3 + (x @ A3e.T) @ B3e.T
        q_e  = silu(w1_e) * (w3_e * ew_e)        (ew commutes into the product)
    out = (sum_e q_e) @ W2.T + sum_e B2e @ (A2e-contraction of q_e)

v3 design (vs v1's identity-broadcast structure; HW exec ~950us vs
v1's ~1.22ms):
  * per-expert w1/w3 live in 4 PSUM banks via DIFF-CHAIN accumulation:
    even banks get CW directly from the big GEMM; odd banks get CW via
    an ACT PSUM->PSUM copy (their has_written bits are seeded once in
    phase 0 and no start=True ever lands on them, so diff packs
    accumulate onto the copied values); each expert-pair pack
    accumulates [B_e ; -B_{e-2}] @ [t_e ; t_{e-2}] (32-row contraction,
    4-way row-group packed) so the previous expert's delta cancels
    exactly.  All 16 identity matmuls of v1 are gone.
  * NO gpsimd tensor ops: concurrent gpsimd TT work starves the DVE's
    SBUF ports and slows vector bf16 ops ~3x (measured); vector PSUM-src
    ops and ACT are immune.  All elementwise runs on vector + ACT.
  * pair-width bf16 DVE ops ([P, 2*NTOK]) amortize the DVE fixed
    overhead; h is a pairwise add tree over the pair tiles.
  * 3 of 8 w3 PSUM-exits are ACT copies (+vector bf16 ew-mult) to
    offload the vector engine, which paces the expert section.
  * software-pipelined emission: iteration i interleaves CW GEMMs of
    tile i with the expert-section packs of tile i-1 and the U-packs of
    tile i-2, so the PE never head-of-line blocks on DVE progress.
  * phase 0: HAM prewarm (dummy matmuls while input DMAs land), bf16
    logits (no fp32 x input at all), lora-down chains overlap the
    softmax, PE-transposes of ew at chain end; identity comes in via
    DMA (make_identity's gpsimd dependency stalls startup).  Dep-free
    junk matmuls pad the main->output transition so the HAM clock gate
    never re-throttles.

Sharding: token-parallel.  Each of the 8 cores gets N/8 = 512 tokens and a
replicated copy of all weights; outputs are disjoint row blocks (no
collectives).  All layout transposes / dtype casts are done host-side.

On-chip layout: feature-on-partition ("transposed"), activations [feat, tok].
"""

import numpy as np

# problem dims (hardcoded per harness contract)
N, D, DFF, E, KTOP, R = 4096, 2048, 8192, 8, 2, 16
NCORES = 8
P = 128

_CACHE = {}


def build_bass(D_=D, DFF_=DFF, E_=E, R_=R, NTOK=N // NCORES, repeat=1):
    """Build the per-core Bass program (same SPMD program on every core)."""
    import concourse.bass as bass
    import concourse.mybir as mybir
    from concourse import bacc
    from concourse.tile import TileContext
    from concourse.masks import make_identity

    dt = mybir.dt
    op = mybir.AluOpType
    AF = mybir.ActivationFunctionType

    KD = D_ // P      # contraction tiles over D
    KF = DFF_ // P    # dff tiles
    MD = D_ // P      # output d tiles
    TT = NTOK // P    # token tiles
    ER = E_ * R_      # stacked expert-rank dim (=128 at full size)
    NCH = 4           # expert-pair packs per tile
    NPRE = 16         # HAM prewarm matmuls

    nc = bacc.Bacc("TRN2", target_bir_lowering=False, debug=False)

    # ---- DRAM I/O ----
    x_bf = nc.dram_tensor("x_bf", [P, KD, NTOK], dt.bfloat16, kind="ExternalInput")
    ident_in = nc.dram_tensor("ident_in", [P, P], dt.float32, kind="ExternalInput")
    gate = nc.dram_tensor("gate", [P, KD, E_], dt.bfloat16, kind="ExternalInput")
    w1t = nc.dram_tensor("w1t", [KF, P, KD, P], dt.bfloat16, kind="ExternalInput")
    w3t = nc.dram_tensor("w3t", [KF, P, KD, P], dt.bfloat16, kind="ExternalInput")
    w2t = nc.dram_tensor("w2t", [MD, P, KF, P], dt.bfloat16, kind="ExternalInput")
    # A pack chains: chain p col slots (16 wide): 0:A1[2p] 16:A1[2p-2]
    # 32:A3[2p] 48:A3[2p-2] 64:A1[2p+1] 80:A1[2p-1] 96:A3[2p+1]
    # 112:A3[2p-1]  (expert indices mod 8)
    amix = nc.dram_tensor("amix", [P, KD, NCH, P], dt.bfloat16, kind="ExternalInput")
    # B diff chains matching amix slots; negative blocks negated host-side,
    # zeroed for p==0
    bmix = nc.dram_tensor("bmix", [KF, P, NCH, P], dt.bfloat16, kind="ExternalInput")
    a2s = nc.dram_tensor("a2s", [KF, P, ER], dt.bfloat16, kind="ExternalInput")
    # B2 stacked to match the U-bank layout: b2s[m, 32j+i, h, n] =
    # B2[4h+j][m*128+n, i] for i<16, zero for i in 16..31
    b2s = nc.dram_tensor("b2s", [MD, P, 2, P], dt.bfloat16, kind="ExternalInput")
    out_t = nc.dram_tensor("out_t", [D_, NTOK], dt.float32, kind="ExternalOutput")

    with TileContext(nc) as tc:
        with (
            tc.tile_pool(name="persist", bufs=1) as persist,
            # stream banks: even (CW via direct GEMM) double-buffered,
            # odd (CW via ident) single-buffered; U accumulators x2
            tc.tile_pool(name="pe1", bufs=2, space="PSUM") as pe1,
            tc.tile_pool(name="pe3", bufs=2, space="PSUM") as pe3,
            tc.tile_pool(name="po1", bufs=1, space="PSUM") as po1,
            tc.tile_pool(name="po3", bufs=1, space="PSUM") as po3,
            tc.tile_pool(name="psum_u", bufs=1, space="PSUM") as ppool_u,
            tc.tile_pool(name="dram", bufs=1, space="DRAM") as dpool,
            tc.tile_pool(name="p0", bufs=4) as p0,
            tc.tile_pool(name="stream", bufs=2) as stream,
        ):
            for _rep in range(repeat):
                # ---------- persistent tiles ----------
                ident_f = persist.tile([P, P], dt.float32)
                nc.sync.dma_start(out=ident_f, in_=ident_in[:, :])
                ident_bf = persist.tile([P, P], dt.bfloat16)
                nc.vector.tensor_copy(ident_bf, ident_f)
                junk_sb = persist.tile([P, NTOK], dt.bfloat16, tag="junk")
                nc.vector.memset(junk_sb, 0.5)

                xbf = persist.tile([P, KD, NTOK], dt.bfloat16)
                nc.sync.dma_start(out=xbf, in_=x_bf[:, :, :])
                gsb = persist.tile([P, KD, E_], dt.bfloat16, tag="gsb")
                nc.sync.dma_start(out=gsb, in_=gate[:, :, :])
                amx = persist.tile([P, KD, NCH, P], dt.bfloat16, tag="amx")
                nc.sync.dma_start(out=amx, in_=amix[:, :, :, :])

                h_big = persist.tile([P, KF, NTOK], dt.bfloat16)
                ewT_sb = persist.tile([E_, NTOK], dt.bfloat16)
                ew_b = []
                for e in range(E_):
                    ewb_t = persist.tile([P, NTOK], dt.bfloat16, tag=f"ewb{e}")
                    ew_b.append(ewb_t)
                tmix = []
                for c in range(NCH):
                    t_t = persist.tile([P, NTOK], dt.bfloat16, tag=f"tmix{c}")
                    tmix.append(t_t)

                # ---------- phase 0 ----------
                # HAM prewarm: junk matmuls while the input DMAs land.
                junk_ps3 = po3.tile([P, NTOK], dt.float32, tag="o3")
                for i in range(3):
                    nc.tensor.matmul(
                        junk_ps3, lhsT=ident_bf, rhs=junk_sb,
                        start=True, stop=True,
                    )
                junk_ps = po1.tile([P, NTOK], dt.float32, tag="o1")
                for i in range(NPRE):
                    nc.tensor.matmul(
                        junk_ps, lhsT=ident_bf, rhs=junk_sb,
                        start=True, stop=True,
                    )

                ew_td = dpool.tile([E_, NTOK], dt.bfloat16)
                # logits (bf16 inputs, fp32 psum) then DVE softmax; the
                # lora-down chains keep the PE busy while softmax runs.
                ewts = []
                for tt in range(TT):
                    lg = pe1.tile([P, E_], dt.float32, tag="e1")
                    for k in range(KD):
                        nc.tensor.matmul(
                            lg,
                            lhsT=xbf[:, k, tt * P:(tt + 1) * P],
                            rhs=gsb[:, k, :],
                            start=(k == 0),
                            stop=(k == KD - 1),
                        )
                    # softmax chain on DVE/ACT for this token tile
                    l_sb = p0.tile([P, E_], dt.float32, tag="lsb")
                    nc.vector.tensor_copy(l_sb, lg)
                    m1 = p0.tile([P, 1], dt.float32, tag="m1")
                    nc.vector.reduce_max(m1, l_sb, axis=mybir.AxisListType.X)
                    nm1 = p0.tile([P, 1], dt.float32, tag="nm1")
                    nc.vector.tensor_scalar_mul(nm1, m1, -1.0)
                    mask1 = p0.tile([P, E_], dt.float32, tag="mask1")
                    nc.vector.tensor_scalar(
                        mask1, l_sb, scalar1=m1, scalar2=None, op0=op.is_equal
                    )
                    l2 = p0.tile([P, E_], dt.float32, tag="l2")
                    nc.vector.scalar_tensor_tensor(
                        l2, in0=mask1, scalar=-1e30, in1=l_sb, op0=op.mult, op1=op.add
                    )
                    m2 = p0.tile([P, 1], dt.float32, tag="m2")
                    nc.vector.reduce_max(m2, l2, axis=mybir.AxisListType.X)
                    mask2 = p0.tile([P, E_], dt.float32, tag="mask2")
                    nc.vector.tensor_scalar(
                        mask2, l2, scalar1=m2, scalar2=None, op0=op.is_equal
                    )
                    mask = p0.tile([P, E_], dt.float32, tag="mask")
                    nc.vector.tensor_tensor(mask, mask1, mask2, op=op.add)
                    pexp = p0.tile([P, E_], dt.float32, tag="pexp")
                    nc.scalar.activation(pexp, l_sb, AF.Exp, bias=nm1, scale=1.0)
                    pm = p0.tile([P, E_], dt.float32, tag="pm")
                    nc.vector.tensor_tensor(pm, pexp, mask, op=op.mult)
                    den = p0.tile([P, 1], dt.float32, tag="den")
                    nc.vector.reduce_sum(den, pm, axis=mybir.AxisListType.X)
                    rec = p0.tile([P, 1], dt.float32, tag="rec")
                    nc.vector.reciprocal(rec, den)
                    ewt = p0.tile([P, E_], dt.float32, tag="ewt")
                    nc.vector.tensor_scalar_mul(ewt, pm, rec)
                    ewts.append(ewt)

                # lora-down chains (PE) overlap the softmax chains (DVE)
                for c in range(NCH):
                    tp = pe3.tile([P, NTOK], dt.float32, tag="e3")
                    for k in range(KD):
                        nc.tensor.matmul(
                            tp,
                            lhsT=amx[:, k, c, :],
                            rhs=xbf[:, k, :],
                            start=(k == 0),
                            stop=(k == KD - 1),
                        )
                    nc.scalar.copy(tmix[c], tp)

                # transpose ew [tok, E] -> [E, tok] on the PE (softmax is
                # done by now), then DRAM roundtrip to broadcast rows.
                for tt in range(TT):
                    ewtp = pe3.tile([E_, P], dt.float32, tag="e3")
                    nc.tensor.transpose(ewtp, ewts[tt], ident_f)
                    nc.scalar.copy(ewT_sb[:, tt * P:(tt + 1) * P], ewtp)
                nc.sync.dma_start(out=ew_td, in_=ewT_sb)
                for e in range(E_):
                    src = bass.AP(
                        tensor=ew_td.tensor,
                        offset=ew_td.offset + e * NTOK,
                        ap=[[0, P], [1, NTOK]],
                    )
                    nc.sync.dma_start(out=ew_b[e], in_=src)

                # ---------- U accumulators ----------
                u_ps_a = ppool_u.tile([P, NTOK], dt.float32, tag="ua")
                u_ps_b = ppool_u.tile([P, NTOK], dt.float32, tag="ub")
                u_ps = [u_ps_a, u_ps_b]
                nc.vector.memset(u_ps_a, 0.0)
                nc.vector.memset(u_ps_b, 0.0)

                # ---------- main loop (software pipelined) ----------
                state = {}

                def u_pack(bank, t):
                    st = state[t]
                    for j in range(4):
                        e = 4 * bank + j
                        qp = st["qp"][2 * bank + j // 2]
                        nc.tensor.matmul(
                            u_ps[bank][32 * j:32 * j + R_, :],
                            lhsT=st["a2"][:, e * R_:(e + 1) * R_],
                            rhs=qp[:, j % 2, :],
                            start=(t == 0),
                            stop=(t == KF - 1),
                            tile_position=(0, 32 * j),
                        )

                def start_tile(t):
                    st = {"qp": {}, "sp": {}, "w3p": {}}
                    state[t] = st
                    w1sl = stream.tile([P, KD, P], dt.bfloat16, tag="w1sl")
                    nc.sync.dma_start(out=w1sl, in_=w1t[t, :, :, :])
                    w3sl = stream.tile([P, KD, P], dt.bfloat16, tag="w3sl")
                    nc.sync.dma_start(out=w3sl, in_=w3t[t, :, :, :])
                    bmk = stream.tile([P, NCH, P], dt.bfloat16, tag="bmk", bufs=2)
                    nc.sync.dma_start(out=bmk, in_=bmix[t, :, :, :])
                    a2kt = stream.tile([P, ER], dt.bfloat16, tag="a2kt", bufs=3)
                    nc.sync.dma_start(out=a2kt, in_=a2s[t, :, :])
                    st["wsl"] = (w1sl, w3sl)
                    st["bmk"] = bmk
                    st["a2"] = a2kt
                    st["banks"] = (
                        pe1.tile([P, NTOK], dt.float32, tag="e1", name="e1b"),
                        pe3.tile([P, NTOK], dt.float32, tag="e3", name="e3b"),
                        po1.tile([P, NTOK], dt.float32, tag="o1", name="o1b"),
                        po3.tile([P, NTOK], dt.float32, tag="o3", name="o3b"),
                    )

                def cw_gemm(t, which, half):
                    st = state[t]
                    tgt = st["banks"][which]
                    src = st["wsl"][which]
                    for k in range(8 * half, 8 * half + 8):
                        nc.tensor.matmul(
                            tgt, lhsT=src[:, k, :], rhs=xbf[:, k, :],
                            start=(k == 0), stop=False,
                            skip_group_check=True,
                        )

                def cw_replicate(t):
                    # ACT copies CW fp32 PSUM->PSUM into the odd banks; the
                    # odd banks' has_written bits were seeded in phase 0 and
                    # never cleared (no start=True ever lands on them), so
                    # the diff packs accumulate onto the copied CW values.
                    st = state[t]
                    nc.scalar.copy(st["banks"][2], st["banks"][0])
                    nc.scalar.copy(st["banks"][3], st["banks"][1])

                def pack(t, p):
                    st = state[t]
                    bmk = st["bmk"]
                    last = (p == NCH - 1)
                    # emit the latest-resolving slot (odd3, gated on vector's
                    # w3q) first so the remaining slots chase it back-to-back
                    # into concurrent row groups
                    for j in (3, 2, 1, 0):
                        tgt = st["banks"][j]
                        r0 = 32 * j
                        nc.tensor.matmul(
                            tgt,
                            lhsT=bmk[r0:r0 + 32, p, :],
                            rhs=tmix[p][r0:r0 + 32, :],
                            start=False, stop=last,
                            tile_position=(r0, 0),
                            skip_group_check=True,
                        )

                # experts 1, 3, 5 get their w3 PSUM-exit via an ACT copy
                # (ACT is immune to engine interference and has slack);
                # the remaining 5 stay as vector PSUM-mults
                ACT_W3 = (1, 3, 5)

                def silu_w3q(t, p):
                    st = state[t]
                    e1b, e3b, o1b, o3b = st["banks"]
                    # pair tiles: bf16 elementwise runs at [P, 2*NTOK] width
                    # so the DVE per-op overhead amortizes 2x while keeping
                    # the FIFO latency quantum small
                    sp = stream.tile(
                        [P, 2, NTOK], dt.bfloat16, tag="sp", bufs=5, name="sp")
                    w3p = stream.tile(
                        [P, 2, NTOK], dt.bfloat16, tag="w3p", bufs=5, name="w3p")
                    st["sp"][p] = sp
                    st["w3p"][p] = w3p
                    # odd expert first: releases the single-buffered odd
                    # banks earliest
                    for ee, w1b, w3b in ((2 * p + 1, o1b, o3b), (2 * p, e1b, e3b)):
                        j = ee % 2
                        nc.scalar.activation(sp[:, j, :], w1b, AF.Silu)
                        if ee in ACT_W3:
                            w3r = stream.tile(
                                [P, NTOK], dt.bfloat16, tag="w3r", bufs=2,
                                name="w3r")
                            nc.scalar.copy(w3r, w3b)
                            nc.vector.tensor_tensor(
                                w3p[:, j, :], w3r, ew_b[ee], op=op.mult)
                        else:
                            nc.vector.tensor_tensor(
                                w3p[:, j, :], w3b, ew_b[ee], op=op.mult)

                def q_pair(t, p):
                    st = state[t]
                    qp = stream.tile(
                        [P, 2, NTOK], dt.bfloat16, tag="qp", bufs=6, name="qp")
                    nc.vector.tensor_tensor(
                        qp, st["sp"][p], st["w3p"][p], op=op.mult)
                    st["qp"][p] = qp

                def h_ops(t, step):
                    st = state[t]
                    if step == 0:
                        v01 = stream.tile(
                            [P, 2, NTOK], dt.bfloat16, tag="v01", bufs=2,
                            name="v01")
                        nc.vector.tensor_tensor(
                            v01, st["qp"][0], st["qp"][1], op=op.add)
                        st["v01"] = v01
                    elif step == 1:
                        v23 = stream.tile(
                            [P, 2, NTOK], dt.bfloat16, tag="v23", bufs=2,
                            name="v23")
                        nc.vector.tensor_tensor(
                            v23, st["qp"][2], st["qp"][3], op=op.add)
                        st["v23"] = v23
                    elif step == 2:
                        vv = stream.tile(
                            [P, 2, NTOK], dt.bfloat16, tag="vv", bufs=2,
                            name="vv")
                        nc.vector.tensor_tensor(
                            vv, st["v01"], st["v23"], op=op.add)
                        st["vv"] = vv
                    else:
                        nc.vector.tensor_tensor(
                            h_big[:, t, :], st["vv"][:, 0, :], st["vv"][:, 1, :],
                            op=op.add)

                def junk_fill(n):
                    # dep-free PE filler across the main->output transition
                    # so the HAM clock gate never sees a >3.4us idle window
                    jt = pe1.tile([P, NTOK], dt.float32, tag="e1", name="jt")
                    for _ in range(n):
                        nc.tensor.matmul(
                            jt, lhsT=ident_bf, rhs=junk_sb,
                            start=True, stop=True,
                        )

                for t in range(KF + 2):
                    a = t - 1   # tile in expert-section stage
                    b = t - 2   # tile in tail/U stage
                    if t < KF:
                        start_tile(t)
                        cw_gemm(t, 0, 0)
                    elif t == KF:
                        junk_fill(3)
                    else:
                        junk_fill(6)
                    if 0 <= a < KF:
                        cw_replicate(a)
                        pack(a, 0)
                        silu_w3q(a, 0)
                    if 0 <= b < KF:
                        q_pair(b, 3)
                        h_ops(b, 1)
                    if t < KF:
                        cw_gemm(t, 0, 1)
                    elif t == KF:
                        junk_fill(4)
                    if 0 <= a < KF:
                        pack(a, 1)
                        silu_w3q(a, 1)
                        q_pair(a, 0)
                    if 0 <= b < KF:
                        h_ops(b, 2)
                        h_ops(b, 3)
                        u_pack(0, b)
                    if t < KF:
                        cw_gemm(t, 1, 0)
                    elif t == KF:
                        junk_fill(4)
                    if 0 <= a < KF:
                        pack(a, 2)
                        silu_w3q(a, 2)
                        q_pair(a, 1)
                        h_ops(a, 0)
                    if t < KF:
                        cw_gemm(t, 1, 1)
                    elif t == KF:
                        junk_fill(4)
                    if 0 <= a < KF:
                        pack(a, 3)
                        silu_w3q(a, 3)
                        q_pair(a, 2)
                    if 0 <= b < KF:
                        u_pack(1, b)
                        del state[b]

                # ---------- export U banks ----------
                uq2 = []
                for bank in range(2):
                    uq_t = persist.tile([P, NTOK], dt.bfloat16, tag=f"uq{bank}")
                    nc.vector.tensor_copy(uq_t, u_ps[bank])
                    uq2.append(uq_t)

                # ---------- output GEMM: out = W2 @ H + B2stack @ Uqall ----------
                # w2m half-slabs are prefetched one step ahead so the PE
                # never waits on the 512KB loads at phase entry.
                KH = KF // 2
                w2q = []
                for h in range(2):
                    w2m = stream.tile(
                        [P, KH, P], dt.bfloat16, tag="w2m", bufs=2, name="w2m")
                    nc.sync.dma_start(out=w2m, in_=w2t[0, :, h * KH:(h + 1) * KH, :])
                    w2q.append(w2m)
                b2q = stream.tile([P, 2, P], dt.bfloat16, tag="b2m", bufs=2, name="b2q")
                nc.sync.dma_start(out=b2q, in_=b2s[0, :, :, :])
                for m in range(MD):
                    outp = pe1.tile([P, NTOK], dt.float32, tag="e1")
                    w2h, b2m = w2q, b2q
                    if m + 1 < MD:
                        w2q = []
                        for h in range(2):
                            w2m = stream.tile(
                                [P, KH, P], dt.bfloat16, tag="w2m", bufs=2,
                                name="w2m")
                            nc.sync.dma_start(
                                out=w2m, in_=w2t[m + 1, :, h * KH:(h + 1) * KH, :])
                            w2q.append(w2m)
                        b2q = stream.tile(
                            [P, 2, P], dt.bfloat16, tag="b2m", bufs=2, name="b2q")
                        nc.sync.dma_start(out=b2q, in_=b2s[m + 1, :, :, :])
                    for h in range(2):
                        for kk in range(KH):
                            kt = h * KH + kk
                            nc.tensor.matmul(
                                outp, lhsT=w2h[h][:, kk, :], rhs=h_big[:, kt, :],
                                start=(kt == 0), stop=False,
                            )
                    nc.tensor.matmul(
                        outp, lhsT=b2m[:, 0, :], rhs=uq2[0], start=False, stop=False,
                    )
                    nc.tensor.matmul(
                        outp, lhsT=b2m[:, 1, :], rhs=uq2[1], start=False, stop=True,
                    )
                    osb = stream.tile([P, NTOK], dt.float32, tag="osb")
                    nc.scalar.copy(osb, outp)
                    nc.sync.dma_start(out=out_t[m * P:(m + 1) * P, :], in_=osb)

    nc.compile()
    return nc


def _sw_d(arr):
    """[D, ...] -> [P, KD, ...] partition-major swizzle (d = k*128 + p)."""
    D_ = arr.shape[0]
    rest = arr.shape[1:]
    return np.ascontiguousarray(
        arr.reshape(D_ // 128, 128, *rest).swapaxes(0, 1)
    )


def _pack_amix(A1, A3):
    """A1/A3 [E, R, D] -> [P, KD, 4, 128] diff chains.

    chain p col slots (16 wide): [A1[2p], A1[2p-2], A3[2p], A3[2p-2],
    A1[2p+1], A1[2p-1], A3[2p+1], A3[2p-1]] (indices mod 8)."""
    E_, R_, D_ = A1.shape
    out = np.zeros((D_, 4, 128), A1.dtype)
    for p in range(4):
        sl = [
            A1[2 * p], A1[(2 * p - 2) % 8], A3[2 * p], A3[(2 * p - 2) % 8],
            A1[2 * p + 1], A1[(2 * p - 1) % 8], A3[2 * p + 1], A3[(2 * p - 1) % 8],
        ]
        for j, A in enumerate(sl):
            out[:, p, 16 * j:16 * j + R_] = A.T
    return _sw_d(out)


def _pack_bmix(B1, B3):
    """B1/B3 [E, F, R] -> [KF, 128, 4, 128] diff chains.

    bmix[kt, r, p, m] rows (16 wide): [+B1[2p], -B1[2p-2], +B3[2p],
    -B3[2p-2], +B1[2p+1], -B1[2p-1], +B3[2p+1], -B3[2p-1]]; the negative
    blocks are zero for p == 0 (each tile's chains restart from CW)."""
    E_, F_, R_ = B1.shape
    out = np.zeros((128, 4, F_), B1.dtype)
    for p in range(4):
        sl = [
            (B1[2 * p], 1.0),
            (B1[(2 * p - 2) % 8], -1.0 if p > 0 else 0.0),
            (B3[2 * p], 1.0),
            (B3[(2 * p - 2) % 8], -1.0 if p > 0 else 0.0),
            (B1[2 * p + 1], 1.0),
            (B1[(2 * p - 1) % 8], -1.0 if p > 0 else 0.0),
            (B3[2 * p + 1], 1.0),
            (B3[(2 * p - 1) % 8], -1.0 if p > 0 else 0.0),
        ]
        for j, (B, sgn) in enumerate(sl):
            if sgn != 0.0:
                out[16 * j:16 * j + R_, p, :] = sgn * B.T
    # [row, p, (kt n)] -> [kt, row, p, n]
    return np.ascontiguousarray(
        out.reshape(128, 4, F_ // 128, 128).transpose(2, 0, 1, 3)
    )


def _pack_b2(B2):
    """B2 [E, D, R] -> [MD, 128, 2, 128] matching the U-bank strip layout.

    out[m, 32j+i, h, n] = B2[4h+j][m*128+n, i] for i<16, zeros at i>=16."""
    E_, D_, R_ = B2.shape
    out = np.zeros((128, 2, D_), B2.dtype)
    for h in range(2):
        for j in range(4):
            out[32 * j:32 * j + R_, h, :] = B2[4 * h + j].T
    # [r, h, (m n)] -> [m, r, h, n]
    return np.ascontiguousarray(
        out.reshape(128, 2, D_ // 128, 128).transpose(2, 0, 1, 3)
    )


def _pack_w_ktiles(WT):
    """WT [K, M] (contraction-major) -> [MT, P, KT, P] where
    out[mt, p, kt, n] = WT[kt*128+p, mt*128+n] — per-(mt) slab is
    partition-major with [KT, 128] contiguous per partition."""
    K_, M_ = WT.shape
    return np.ascontiguousarray(
        WT.reshape(K_ // 128, 128, M_ // 128, 128).transpose(2, 1, 0, 3)
    )


def _prep_inputs(x, W1, W3, W2, gate_w, A1, B1, A3, B3, A2, B2):
    """Host-side packing: transposes + casts, shared across cores."""
    import ml_dtypes

    bf16 = ml_dtypes.bfloat16
    f32 = np.float32

    xT = np.ascontiguousarray(np.asarray(x, f32).T)            # [D, N]
    dff = W1.shape[0]
    shared = {
        "gate": _sw_d(np.ascontiguousarray(np.asarray(gate_w, f32).T)).astype(bf16),
        "w1t": _pack_w_ktiles(np.asarray(W1, f32).T.astype(bf16)),
        "w3t": _pack_w_ktiles(np.asarray(W3, f32).T.astype(bf16)),
        "w2t": _pack_w_ktiles(np.asarray(W2, f32).T.astype(bf16)),
        "amix": _pack_amix(np.asarray(A1, f32), np.asarray(A3, f32)).astype(bf16),
        "bmix": _pack_bmix(np.asarray(B1, f32), np.asarray(B3, f32)).astype(bf16),
        "a2s": np.ascontiguousarray(
            np.asarray(A2, f32).transpose(2, 0, 1).reshape(dff // 128, 128, -1)
        ).astype(bf16),
        "b2s": _pack_b2(np.asarray(B2, f32)).astype(bf16),
    }
    shared["ident_in"] = np.eye(128, dtype=f32)
    ntok = xT.shape[1] // NCORES
    in_maps = []
    for c in range(NCORES):
        sl = np.ascontiguousarray(xT[:, c * ntok:(c + 1) * ntok])
        m = dict(shared)
        m["x_bf"] = _sw_d(sl.astype(bf16))
        in_maps.append(m)
    return in_maps


def _ensure_compiled():
    if "exec" not in _CACHE:
        nc = build_bass()
        _CACHE["nc"] = nc
        _CACHE["exec"] = _make_exec(nc)
    return _CACHE["exec"]


def _make_exec(nc):
    """Build a jitted 8-core shard_map executor for a Bass program.

    Mirrors concourse.bass2jax.run_bass_via_pjrt, but caches the jitted
    callable and keeps real inputs un-donated so device buffers can be
    reused across calls (for timing)."""
    import jax
    import concourse.mybir as mybir
    from concourse import bass2jax
    from jax.experimental.shard_map import shard_map
    from jax.sharding import Mesh, PartitionSpec

    bass2jax.install_neuronx_cc_hook()

    partition_name = (
        nc.partition_id_tensor.name if nc.partition_id_tensor else None
    )
    in_names, out_names, out_avals, zero_outs = [], [], [], []
    for alloc in nc.m.functions[0].allocations:
        if not isinstance(alloc, mybir.MemoryLocationSet):
            continue
        name = alloc.memorylocations[0].name
        if alloc.kind == "ExternalInput":
            if name != partition_name:
                in_names.append(name)
        elif alloc.kind == "ExternalOutput":
            np_dtype = mybir.dt.np(alloc.dtype)
            out_names.append(name)
            out_avals.append(
                jax.core.ShapedArray(tuple(alloc.tensor_shape), np_dtype)
            )
            zero_outs.append(np.zeros(tuple(alloc.tensor_shape), np_dtype))

    n_params = len(in_names)
    n_outs = len(out_names)
    all_names = in_names + out_names
    if partition_name is not None:
        all_names = all_names + [partition_name]

    def _body(*args):
        operands = list(args)
        if partition_name is not None:
            operands.append(bass2jax.partition_id_tensor())
        outs = bass2jax._bass_exec_p.bind(
            *operands,
            out_avals=tuple(out_avals),
            in_names=tuple(all_names),
            out_names=tuple(out_names),
            lowering_input_output_aliases=(),
            sim_require_finite=True,
            sim_require_nnan=True,
            nc=nc,
        )
        return tuple(outs)

    devices = jax.devices()[:NCORES]
    mesh = Mesh(np.asarray(devices), ("core",))
    in_specs = (PartitionSpec("core"),) * (n_params + n_outs)
    out_specs = (PartitionSpec("core"),) * n_outs
    donate = tuple(range(n_params, n_params + n_outs))
    sharded = jax.jit(
        shard_map(
            _body, mesh=mesh, in_specs=in_specs, out_specs=out_specs,
            check_rep=False,
        ),
        donate_argnums=donate,
        keep_unused=True,
    )
    ctx = {
        "fn": sharded,
        "body": _body,
        "n_operands": n_params + n_outs,
        "in_names": in_names,
        "out_names": out_names,
        "zero_outs": zero_outs,
        "mesh": mesh,
    }
    return ctx


def _concat_inputs(in_maps, in_names):
    return [
        np.concatenate([in_maps[c][nm] for c in range(NCORES)], axis=0)
        for nm in in_names
    ]


def _run(ctx, concat_in):
    zeros = [
        np.zeros((NCORES * z.shape[0], *z.shape[1:]), z.dtype)
        for z in ctx["zero_outs"]
    ]
    return ctx["fn"](*concat_in, *zeros)


def kernel(x, W1, W3, W2, gate_w, A1, B1, A3, B3, A2, B2):
    ctx = _ensure_compiled()
    in_maps = _prep_inputs(x, W1, W3, W2, gate_w, A1, B1, A3, B3, A2, B2)
    concat_in = _concat_inputs(in_maps, ctx["in_names"])
    out_arrs = _run(ctx, concat_in)
    ntok = N // NCORES
    res = np.asarray(out_arrs[ctx["out_names"].index("out_t")])
    res = res.reshape(NCORES, D, ntok)
    out = np.empty((N, D), np.float32)
    for c in range(NCORES):
        out[c * ntok:(c + 1) * ntok, :] = res[c].T
    return out


def time_device(inputs, iters=3, ctx=None):
    """Upload all operands once (no donation), then wall-time jitted runs."""
    import time as _time

    import jax
    from jax.experimental.shard_map import shard_map
    from jax.sharding import NamedSharding, PartitionSpec, Mesh

    if ctx is None:
        ctx = _ensure_compiled()
    if "fn_nodonate" not in ctx:
        ctx["fn_nodonate"] = jax.jit(
            shard_map(
                ctx["body"], mesh=ctx["mesh"],
                in_specs=(PartitionSpec("core"),) * ctx["n_operands"],
                out_specs=(PartitionSpec("core"),) * len(ctx["out_names"]),
                check_rep=False,
            ),
            keep_unused=True,
        )
    fn = ctx["fn_nodonate"]
    in_maps = _prep_inputs(**inputs)
    concat_in = _concat_inputs(in_maps, ctx["in_names"])
    zeros = [
        np.zeros((NCORES * z.shape[0], *z.shape[1:]), z.dtype)
        for z in ctx["zero_outs"]
    ]
    sh = NamedSharding(ctx["mesh"], PartitionSpec("core"))
    dev = [jax.device_put(a, sh) for a in (concat_in + zeros)]
    jax.block_until_ready(fn(*dev))  # warmup/compile
    times = []
    for _ in range(iters):
        t0 = _time.perf_counter()
        jax.block_until_ready(fn(*dev))
        times.append(_time.perf_counter() - t0)
    return min(times)


# revision 30
# speedup vs baseline: 1.0113x; 1.0096x over previous
"""MixFFN MoE-routing kernel for Trainium2 (8 NeuronCores, token-parallel).

Math (per token block):
    logits = x @ gate_w.T ; probs = softmax(logits); top2 -> ew [N, E] (dense, rows sum to 1)
    CW1 = x @ W1.T ; CW3 = x @ W3.T
    per expert e:
        w1_e = CW1 + (x @ A1e.T) @ B1e.T
        w3_e = CW3 + (x @ A3e.T) @ B3e.T
        q_e  = silu(w1_e) * (w3_e * ew_e)        (ew commutes into the product)
    out = (sum_e q_e) @ W2.T + sum_e B2e @ (A2e-contraction of q_e)

v3 design (vs v1's identity-broadcast structure; HW exec ~950us vs
v1's ~1.22ms):
  * per-expert w1/w3 live in 4 PSUM banks via DIFF-CHAIN accumulation:
    even banks get CW directly from the big GEMM; odd banks get CW via
    an ACT PSUM->PSUM copy (their has_written bits are seeded once in
    phase 0 and no start=True ever lands on them, so diff packs
    accumulate onto the copied values); each expert-pair pack
    accumulates [B_e ; -B_{e-2}] @ [t_e ; t_{e-2}] (32-row contraction,
    4-way row-group packed) so the previous expert's delta cancels
    exactly.  All 16 identity matmuls of v1 are gone.
  * NO gpsimd tensor ops: concurrent gpsimd TT work starves the DVE's
    SBUF ports and slows vector bf16 ops ~3x (measured); vector PSUM-src
    ops and ACT are immune.  All elementwise runs on vector + ACT.
  * pair-width bf16 DVE ops ([P, 2*NTOK]) amortize the DVE fixed
    overhead; h is a pairwise add tree over the pair tiles.
  * 3 of 8 w3 PSUM-exits are ACT copies (+vector bf16 ew-mult) to
    offload the vector engine, which paces the expert section.
  * software-pipelined emission: iteration i interleaves CW GEMMs of
    tile i with the expert-section packs of tile i-1 and the U-packs of
    tile i-2, so the PE never head-of-line blocks on DVE progress.
  * phase 0: HAM prewarm (dummy matmuls while input DMAs land), bf16
    logits (no fp32 x input at all), lora-down chains overlap the
    softmax, PE-transposes of ew at chain end; identity comes in via
    DMA (make_identity's gpsimd dependency stalls startup).  Dep-free
    junk matmuls pad the main->output transition so the HAM clock gate
    never re-throttles.

Sharding: token-parallel.  Each of the 8 cores gets N/8 = 512 tokens and a
replicated copy of all weights; outputs are disjoint row blocks (no
collectives).  All layout transposes / dtype casts are done host-side.

On-chip layout: feature-on-partition ("transposed"), activations [feat, tok].
"""

import numpy as np

# problem dims (hardcoded per harness contract)
N, D, DFF, E, KTOP, R = 4096, 2048, 8192, 8, 2, 16
NCORES = 8
P = 128

_CACHE = {}


def build_bass(D_=D, DFF_=DFF, E_=E, R_=R, NTOK=N // NCORES, repeat=1):
    """Build the per-core Bass program (same SPMD program on every core)."""
    import concourse.bass as bass
    import concourse.mybir as mybir
    from concourse import bacc
    from concourse.tile import TileContext
    from concourse.masks import make_identity

    dt = mybir.dt
    op = mybir.AluOpType
    AF = mybir.ActivationFunctionType

    KD = D_ // P      # contraction tiles over D
    KF = DFF_ // P    # dff tiles
    MD = D_ // P      # output d tiles
    TT = NTOK // P    # token tiles
    ER = E_ * R_      # stacked expert-rank dim (=128 at full size)
    NCH = 4           # expert-pair packs per tile
    NPRE = 30         # HAM prewarm matmuls

    nc = bacc.Bacc("TRN2", target_bir_lowering=False, debug=False)

    # ---- DRAM I/O ----
    x_bf = nc.dram_tensor("x_bf", [P, KD, NTOK], dt.bfloat16, kind="ExternalInput")
    ident_in = nc.dram_tensor("ident_in", [P, P], dt.float32, kind="ExternalInput")
    gate = nc.dram_tensor("gate", [P, KD, E_], dt.bfloat16, kind="ExternalInput")
    w1t = nc.dram_tensor("w1t", [KF, P, KD, P], dt.bfloat16, kind="ExternalInput")
    w3t = nc.dram_tensor("w3t", [KF, P, KD, P], dt.bfloat16, kind="ExternalInput")
    w2t = nc.dram_tensor("w2t", [MD, P, KF, P], dt.bfloat16, kind="ExternalInput")
    # A pack chains: chain p col slots (16 wide): 0:A1[2p] 16:A1[2p-2]
    # 32:A3[2p] 48:A3[2p-2] 64:A1[2p+1] 80:A1[2p-1] 96:A3[2p+1]
    # 112:A3[2p-1]  (expert indices mod 8)
    amix = nc.dram_tensor("amix", [P, KD, NCH, P], dt.bfloat16, kind="ExternalInput")
    # B diff chains matching amix slots; negative blocks negated host-side,
    # zeroed for p==0
    bmix = nc.dram_tensor("bmix", [KF, P, NCH, P], dt.bfloat16, kind="ExternalInput")
    a2s = nc.dram_tensor("a2s", [KF, P, ER], dt.bfloat16, kind="ExternalInput")
    # B2 stacked to match the U-bank layout: b2s[m, 32j+i, h, n] =
    # B2[4h+j][m*128+n, i] for i<16, zero for i in 16..31
    b2s = nc.dram_tensor("b2s", [MD, P, 2, P], dt.bfloat16, kind="ExternalInput")
    out_t = nc.dram_tensor("out_t", [D_, NTOK], dt.float32, kind="ExternalOutput")

    with TileContext(nc) as tc:
        with (
            tc.tile_pool(name="persist", bufs=1) as persist,
            # stream banks: even (CW via direct GEMM) double-buffered,
            # odd (CW via ident) single-buffered; U accumulators x2
            tc.tile_pool(name="pe1", bufs=2, space="PSUM") as pe1,
            tc.tile_pool(name="pe3", bufs=2, space="PSUM") as pe3,
            tc.tile_pool(name="po1", bufs=1, space="PSUM") as po1,
            tc.tile_pool(name="po3", bufs=1, space="PSUM") as po3,
            tc.tile_pool(name="psum_u", bufs=1, space="PSUM") as ppool_u,
            tc.tile_pool(name="dram", bufs=1, space="DRAM") as dpool,
            tc.tile_pool(name="p0", bufs=4) as p0,
            tc.tile_pool(name="stream", bufs=2) as stream,
        ):
            for _rep in range(repeat):
                # ---------- persistent tiles ----------
                ident_f = persist.tile([P, P], dt.float32)
                nc.sync.dma_start(out=ident_f, in_=ident_in[:, :])
                ident_bf = persist.tile([P, P], dt.bfloat16)
                nc.vector.tensor_copy(ident_bf, ident_f)
                junk_sb = persist.tile([P, NTOK], dt.bfloat16, tag="junk")
                nc.vector.memset(junk_sb, 0.5)

                xbf = persist.tile([P, KD, NTOK], dt.bfloat16)
                nc.sync.dma_start(out=xbf, in_=x_bf[:, :, :])
                gsb = persist.tile([P, KD, E_], dt.bfloat16, tag="gsb")
                nc.sync.dma_start(out=gsb, in_=gate[:, :, :])
                amx = persist.tile([P, KD, NCH, P], dt.bfloat16, tag="amx")
                nc.sync.dma_start(out=amx, in_=amix[:, :, :, :])

                h_big = persist.tile([P, KF, NTOK], dt.bfloat16)
                ewT_sb = persist.tile([E_, NTOK], dt.bfloat16)
                ew_b = []
                for e in range(E_):
                    ewb_t = persist.tile([P, NTOK], dt.bfloat16, tag=f"ewb{e}")
                    ew_b.append(ewb_t)
                tmix = []
                for c in range(NCH):
                    t_t = persist.tile([P, NTOK], dt.bfloat16, tag=f"tmix{c}")
                    tmix.append(t_t)

                # ---------- phase 0 ----------
                # HAM prewarm: junk matmuls while the input DMAs land.
                junk_ps3 = po3.tile([P, NTOK], dt.float32, tag="o3")
                for i in range(3):
                    nc.tensor.matmul(
                        junk_ps3, lhsT=ident_bf, rhs=junk_sb,
                        start=True, stop=True,
                    )
                junk_ps = po1.tile([P, NTOK], dt.float32, tag="o1")
                for i in range(NPRE):
                    nc.tensor.matmul(
                        junk_ps, lhsT=ident_bf, rhs=junk_sb,
                        start=True, stop=True,
                    )

                ew_td = dpool.tile([E_, NTOK], dt.bfloat16)
                # logits (bf16 inputs, fp32 psum) then DVE softmax; the
                # lora-down chains keep the PE busy while softmax runs.
                ewts = []
                for tt in range(TT):
                    lg = pe1.tile([P, E_], dt.float32, tag="e1")
                    for k in range(KD):
                        nc.tensor.matmul(
                            lg,
                            lhsT=xbf[:, k, tt * P:(tt + 1) * P],
                            rhs=gsb[:, k, :],
                            start=(k == 0),
                            stop=(k == KD - 1),
                        )
                    # softmax chain on DVE/ACT for this token tile
                    l_sb = p0.tile([P, E_], dt.float32, tag="lsb")
                    nc.vector.tensor_copy(l_sb, lg)
                    m1 = p0.tile([P, 1], dt.float32, tag="m1")
                    nc.vector.reduce_max(m1, l_sb, axis=mybir.AxisListType.X)
                    nm1 = p0.tile([P, 1], dt.float32, tag="nm1")
                    nc.vector.tensor_scalar_mul(nm1, m1, -1.0)
                    mask1 = p0.tile([P, E_], dt.float32, tag="mask1")
                    nc.vector.tensor_scalar(
                        mask1, l_sb, scalar1=m1, scalar2=None, op0=op.is_equal
                    )
                    l2 = p0.tile([P, E_], dt.float32, tag="l2")
                    nc.vector.scalar_tensor_tensor(
                        l2, in0=mask1, scalar=-1e30, in1=l_sb, op0=op.mult, op1=op.add
                    )
                    m2 = p0.tile([P, 1], dt.float32, tag="m2")
                    nc.vector.reduce_max(m2, l2, axis=mybir.AxisListType.X)
                    mask2 = p0.tile([P, E_], dt.float32, tag="mask2")
                    nc.vector.tensor_scalar(
                        mask2, l2, scalar1=m2, scalar2=None, op0=op.is_equal
                    )
                    mask = p0.tile([P, E_], dt.float32, tag="mask")
                    nc.vector.tensor_tensor(mask, mask1, mask2, op=op.add)
                    pexp = p0.tile([P, E_], dt.float32, tag="pexp")
                    nc.scalar.activation(pexp, l_sb, AF.Exp, bias=nm1, scale=1.0)
                    pm = p0.tile([P, E_], dt.float32, tag="pm")
                    nc.vector.tensor_tensor(pm, pexp, mask, op=op.mult)
                    den = p0.tile([P, 1], dt.float32, tag="den")
                    nc.vector.reduce_sum(den, pm, axis=mybir.AxisListType.X)
                    rec = p0.tile([P, 1], dt.float32, tag="rec")
                    nc.vector.reciprocal(rec, den)
                    ewt = p0.tile([P, E_], dt.float32, tag="ewt")
                    nc.vector.tensor_scalar_mul(ewt, pm, rec)
                    ewts.append(ewt)

                # lora-down chains (PE) overlap the softmax chains (DVE)
                for c in range(NCH):
                    tp = pe3.tile([P, NTOK], dt.float32, tag="e3")
                    for k in range(KD):
                        nc.tensor.matmul(
                            tp,
                            lhsT=amx[:, k, c, :],
                            rhs=xbf[:, k, :],
                            start=(k == 0),
                            stop=(k == KD - 1),
                        )
                    nc.scalar.copy(tmix[c], tp)

                # transpose ew [tok, E] -> [E, tok] on the PE (softmax is
                # done by now), then DRAM roundtrip to broadcast rows.
                for tt in range(TT):
                    ewtp = pe3.tile([E_, P], dt.float32, tag="e3")
                    nc.tensor.transpose(ewtp, ewts[tt], ident_f)
                    nc.scalar.copy(ewT_sb[:, tt * P:(tt + 1) * P], ewtp)
                nc.sync.dma_start(out=ew_td, in_=ewT_sb)
                for e in range(E_):
                    src = bass.AP(
                        tensor=ew_td.tensor,
                        offset=ew_td.offset + e * NTOK,
                        ap=[[0, P], [1, NTOK]],
                    )
                    nc.sync.dma_start(out=ew_b[e], in_=src)

                # ---------- U accumulators ----------
                u_ps_a = ppool_u.tile([P, NTOK], dt.float32, tag="ua")
                u_ps_b = ppool_u.tile([P, NTOK], dt.float32, tag="ub")
                u_ps = [u_ps_a, u_ps_b]
                nc.vector.memset(u_ps_a, 0.0)
                nc.vector.memset(u_ps_b, 0.0)

                # ---------- main loop (software pipelined) ----------
                state = {}

                def u_pack(bank, t):
                    st = state[t]
                    for j in range(4):
                        e = 4 * bank + j
                        qp = st["qp"][2 * bank + j // 2]
                        nc.tensor.matmul(
                            u_ps[bank][32 * j:32 * j + R_, :],
                            lhsT=st["a2"][:, e * R_:(e + 1) * R_],
                            rhs=qp[:, j % 2, :],
                            start=(t == 0),
                            stop=(t == KF - 1),
                            tile_position=(0, 32 * j),
                        )

                def start_tile(t):
                    st = {"qp": {}, "sp": {}, "w3p": {}}
                    state[t] = st
                    w1sl = stream.tile([P, KD, P], dt.bfloat16, tag="w1sl")
                    nc.sync.dma_start(out=w1sl, in_=w1t[t, :, :, :])
                    w3sl = stream.tile([P, KD, P], dt.bfloat16, tag="w3sl")
                    nc.sync.dma_start(out=w3sl, in_=w3t[t, :, :, :])
                    bmk = stream.tile([P, NCH, P], dt.bfloat16, tag="bmk", bufs=2)
                    nc.sync.dma_start(out=bmk, in_=bmix[t, :, :, :])
                    a2kt = stream.tile([P, ER], dt.bfloat16, tag="a2kt", bufs=3)
                    nc.sync.dma_start(out=a2kt, in_=a2s[t, :, :])
                    st["wsl"] = (w1sl, w3sl)
                    st["bmk"] = bmk
                    st["a2"] = a2kt
                    st["banks"] = (
                        pe1.tile([P, NTOK], dt.float32, tag="e1", name="e1b"),
                        pe3.tile([P, NTOK], dt.float32, tag="e3", name="e3b"),
                        po1.tile([P, NTOK], dt.float32, tag="o1", name="o1b"),
                        po3.tile([P, NTOK], dt.float32, tag="o3", name="o3b"),
                    )

                def cw_gemm(t, which, half):
                    st = state[t]
                    tgt = st["banks"][which]
                    src = st["wsl"][which]
                    for k in range(8 * half, 8 * half + 8):
                        nc.tensor.matmul(
                            tgt, lhsT=src[:, k, :], rhs=xbf[:, k, :],
                            start=(k == 0), stop=False,
                            skip_group_check=True,
                        )

                def cw_replicate(t):
                    # ACT copies CW fp32 PSUM->PSUM into the odd banks; the
                    # odd banks' has_written bits were seeded in phase 0 and
                    # never cleared (no start=True ever lands on them), so
                    # the diff packs accumulate onto the copied CW values.
                    st = state[t]
                    nc.scalar.copy(st["banks"][2], st["banks"][0])
                    nc.scalar.copy(st["banks"][3], st["banks"][1])

                def pack(t, p):
                    st = state[t]
                    bmk = st["bmk"]
                    last = (p == NCH - 1)
                    # emit the latest-resolving slot (odd3, gated on vector's
                    # w3q) first so the remaining slots chase it back-to-back
                    # into concurrent row groups
                    for j in (3, 2, 1, 0):
                        tgt = st["banks"][j]
                        r0 = 32 * j
                        nc.tensor.matmul(
                            tgt,
                            lhsT=bmk[r0:r0 + 32, p, :],
                            rhs=tmix[p][r0:r0 + 32, :],
                            start=False, stop=last,
                            tile_position=(r0, 0),
                            skip_group_check=True,
                        )

                # experts 1, 3, 5 get their w3 PSUM-exit via an ACT copy
                # (ACT is immune to engine interference and has slack);
                # the remaining 5 stay as vector PSUM-mults
                ACT_W3 = (1, 3, 5)

                def silu_w3q(t, p):
                    st = state[t]
                    e1b, e3b, o1b, o3b = st["banks"]
                    # pair tiles: bf16 elementwise runs at [P, 2*NTOK] width
                    # so the DVE per-op overhead amortizes 2x while keeping
                    # the FIFO latency quantum small
                    sp = stream.tile(
                        [P, 2, NTOK], dt.bfloat16, tag="sp", bufs=5, name="sp")
                    w3p = stream.tile(
                        [P, 2, NTOK], dt.bfloat16, tag="w3p", bufs=5, name="w3p")
                    st["sp"][p] = sp
                    st["w3p"][p] = w3p
                    # odd expert first: releases the single-buffered odd
                    # banks earliest
                    for ee, w1b, w3b in ((2 * p + 1, o1b, o3b), (2 * p, e1b, e3b)):
                        j = ee % 2
                        nc.scalar.activation(sp[:, j, :], w1b, AF.Silu)
                        if ee in ACT_W3:
                            w3r = stream.tile(
                                [P, NTOK], dt.bfloat16, tag="w3r", bufs=2,
                                name="w3r")
                            nc.scalar.copy(w3r, w3b)
                            nc.vector.tensor_tensor(
                                w3p[:, j, :], w3r, ew_b[ee], op=op.mult)
                        else:
                            nc.vector.tensor_tensor(
                                w3p[:, j, :], w3b, ew_b[ee], op=op.mult)

                def q_pair(t, p):
                    st = state[t]
                    qp = stream.tile(
                        [P, 2, NTOK], dt.bfloat16, tag="qp", bufs=6, name="qp")
                    nc.vector.tensor_tensor(
                        qp, st["sp"][p], st["w3p"][p], op=op.mult)
                    st["qp"][p] = qp

                def h_ops(t, step):
                    st = state[t]
                    if step == 0:
                        v01 = stream.tile(
                            [P, 2, NTOK], dt.bfloat16, tag="v01", bufs=2,
                            name="v01")
                        nc.vector.tensor_tensor(
                            v01, st["qp"][0], st["qp"][1], op=op.add)
                        st["v01"] = v01
                    elif step == 1:
                        v23 = stream.tile(
                            [P, 2, NTOK], dt.bfloat16, tag="v23", bufs=2,
                            name="v23")
                        nc.vector.tensor_tensor(
                            v23, st["qp"][2], st["qp"][3], op=op.add)
                        st["v23"] = v23
                    elif step == 2:
                        vv = stream.tile(
                            [P, 2, NTOK], dt.bfloat16, tag="vv", bufs=2,
                            name="vv")
                        nc.vector.tensor_tensor(
                            vv, st["v01"], st["v23"], op=op.add)
                        st["vv"] = vv
                    else:
                        nc.vector.tensor_tensor(
                            h_big[:, t, :], st["vv"][:, 0, :], st["vv"][:, 1, :],
                            op=op.add)

                # m=0 of the output GEMM starts inside the epilogue: its
                # W2 matmuls (whose h tiles are long ready) fill the PE-FIFO
                # gaps behind the DVE-gated final packs, replacing junk
                # filler and keeping the HAM clock gate warm.
                KH = KF // 2
                out_cur = {"outp": None, "w2h": None, "kt": 0}

                def emit_out_mms(n):
                    oc = out_cur
                    if oc["outp"] is None:
                        oc["outp"] = pe1.tile(
                            [P, NTOK], dt.float32, tag="e1", name="outp0")
                    for _ in range(n):
                        kt = oc["kt"]
                        if kt >= KF - 2:
                            return
                        nc.tensor.matmul(
                            oc["outp"], lhsT=oc["w2h"][kt // KH][:, kt % KH, :],
                            rhs=h_big[:, kt, :], start=(kt == 0), stop=False,
                        )
                        oc["kt"] += 1

                for t in range(KF + 2):
                    a = t - 1   # tile in expert-section stage
                    b = t - 2   # tile in tail/U stage
                    if t == KF - 2:
                        w2h0 = []
                        for h in range(2):
                            w2m = stream.tile(
                                [P, KH, P], dt.bfloat16, tag="w2m", bufs=2,
                                name="w2m")
                            nc.sync.dma_start(
                                out=w2m, in_=w2t[0, :, h * KH:(h + 1) * KH, :])
                            w2h0.append(w2m)
                        out_cur["w2h"] = w2h0
                    if t < KF:
                        start_tile(t)
                        cw_gemm(t, 0, 0)
                    elif t == KF:
                        emit_out_mms(4)
                    else:
                        emit_out_mms(10)
                    if 0 <= a < KF:
                        cw_replicate(a)
                        pack(a, 0)
                        silu_w3q(a, 0)
                    if 0 <= b < KF:
                        q_pair(b, 3)
                        h_ops(b, 1)
                    if t < KF:
                        cw_gemm(t, 0, 1)
                    elif t == KF:
                        emit_out_mms(8)
                    if 0 <= a < KF:
                        pack(a, 1)
                        silu_w3q(a, 1)
                        q_pair(a, 0)
                    if 0 <= b < KF:
                        h_ops(b, 2)
                        h_ops(b, 3)
                        u_pack(0, b)
                    if t < KF:
                        cw_gemm(t, 1, 0)
                    elif t == KF:
                        emit_out_mms(8)
                    if 0 <= a < KF:
                        pack(a, 2)
                        silu_w3q(a, 2)
                        q_pair(a, 1)
                        h_ops(a, 0)
                    if t < KF:
                        cw_gemm(t, 1, 1)
                    elif t == KF:
                        emit_out_mms(8)
                    if 0 <= a < KF:
                        pack(a, 3)
                        silu_w3q(a, 3)
                        q_pair(a, 2)
                    if 0 <= b < KF:
                        u_pack(1, b)
                        del state[b]

                # ---------- export U banks ----------
                uq2 = []
                for bank in range(2):
                    uq_t = persist.tile([P, NTOK], dt.bfloat16, tag=f"uq{bank}")
                    nc.vector.tensor_copy(uq_t, u_ps[bank])
                    uq2.append(uq_t)

                # ---------- output GEMM: out = W2 @ H + B2stack @ Uqall ----------
                # w2m half-slabs are prefetched one step ahead so the PE
                # never waits on the 512KB loads at phase entry.
                b2q = stream.tile([P, 2, P], dt.bfloat16, tag="b2m", bufs=2, name="b2q")
                nc.sync.dma_start(out=b2q, in_=b2s[0, :, :, :])
                w2q = out_cur["w2h"]
                for m in range(MD):
                    if m == 0:
                        outp = out_cur["outp"]
                        kt_lo = out_cur["kt"]
                    else:
                        outp = pe1.tile([P, NTOK], dt.float32, tag="e1")
                        kt_lo = 0
                    w2h, b2m = w2q, b2q
                    if m + 1 < MD:
                        w2q = []
                        for h in range(2):
                            w2m = stream.tile(
                                [P, KH, P], dt.bfloat16, tag="w2m", bufs=2,
                                name="w2m")
                            nc.sync.dma_start(
                                out=w2m, in_=w2t[m + 1, :, h * KH:(h + 1) * KH, :])
                            w2q.append(w2m)
                        b2q = stream.tile(
                            [P, 2, P], dt.bfloat16, tag="b2m", bufs=2, name="b2q")
                        nc.sync.dma_start(out=b2q, in_=b2s[m + 1, :, :, :])
                    for kt in range(kt_lo, KF):
                        nc.tensor.matmul(
                            outp, lhsT=w2h[kt // KH][:, kt % KH, :],
                            rhs=h_big[:, kt, :],
                            start=(kt == 0), stop=False,
                        )
                    nc.tensor.matmul(
                        outp, lhsT=b2m[:, 0, :], rhs=uq2[0], start=False, stop=False,
                    )
                    nc.tensor.matmul(
                        outp, lhsT=b2m[:, 1, :], rhs=uq2[1], start=False, stop=True,
                    )
                    osb = stream.tile([P, NTOK], dt.float32, tag="osb")
                    nc.scalar.copy(osb, outp)
                    nc.sync.dma_start(out=out_t[m * P:(m + 1) * P, :], in_=osb)

    nc.compile()
    return nc


def _sw_d(arr):
    """[D, ...] -> [P, KD, ...] partition-major swizzle (d = k*128 + p)."""
    D_ = arr.shape[0]
    rest = arr.shape[1:]
    return np.ascontiguousarray(
        arr.reshape(D_ // 128, 128, *rest).swapaxes(0, 1)
    )


def _pack_amix(A1, A3):
    """A1/A3 [E, R, D] -> [P, KD, 4, 128] diff chains.

    chain p col slots (16 wide): [A1[2p], A1[2p-2], A3[2p], A3[2p-2],
    A1[2p+1], A1[2p-1], A3[2p+1], A3[2p-1]] (indices mod 8)."""
    E_, R_, D_ = A1.shape
    out = np.zeros((D_, 4, 128), A1.dtype)
    for p in range(4):
        sl = [
            A1[2 * p], A1[(2 * p - 2) % 8], A3[2 * p], A3[(2 * p - 2) % 8],
            A1[2 * p + 1], A1[(2 * p - 1) % 8], A3[2 * p + 1], A3[(2 * p - 1) % 8],
        ]
        for j, A in enumerate(sl):
            out[:, p, 16 * j:16 * j + R_] = A.T
    return _sw_d(out)


def _pack_bmix(B1, B3):
    """B1/B3 [E, F, R] -> [KF, 128, 4, 128] diff chains.

    bmix[kt, r, p, m] rows (16 wide): [+B1[2p], -B1[2p-2], +B3[2p],
    -B3[2p-2], +B1[2p+1], -B1[2p-1], +B3[2p+1], -B3[2p-1]]; the negative
    blocks are zero for p == 0 (each tile's chains restart from CW)."""
    E_, F_, R_ = B1.shape
    out = np.zeros((128, 4, F_), B1.dtype)
    for p in range(4):
        sl = [
            (B1[2 * p], 1.0),
            (B1[(2 * p - 2) % 8], -1.0 if p > 0 else 0.0),
            (B3[2 * p], 1.0),
            (B3[(2 * p - 2) % 8], -1.0 if p > 0 else 0.0),
            (B1[2 * p + 1], 1.0),
            (B1[(2 * p - 1) % 8], -1.0 if p > 0 else 0.0),
            (B3[2 * p + 1], 1.0),
            (B3[(2 * p - 1) % 8], -1.0 if p > 0 else 0.0),
        ]
        for j, (B, sgn) in enumerate(sl):
            if sgn != 0.0:
                out[16 * j:16 * j + R_, p, :] = sgn * B.T
    # [row, p, (kt n)] -> [kt, row, p, n]
    return np.ascontiguousarray(
        out.reshape(128, 4, F_ // 128, 128).transpose(2, 0, 1, 3)
    )


def _pack_b2(B2):
    """B2 [E, D, R] -> [MD, 128, 2, 128] matching the U-bank strip layout.

    out[m, 32j+i, h, n] = B2[4h+j][m*128+n, i] for i<16, zeros at i>=16."""
    E_, D_, R_ = B2.shape
    out = np.zeros((128, 2, D_), B2.dtype)
    for h in range(2):
        for j in range(4):
            out[32 * j:32 * j + R_, h, :] = B2[4 * h + j].T
    # [r, h, (m n)] -> [m, r, h, n]
    return np.ascontiguousarray(
        out.reshape(128, 2, D_ // 128, 128).transpose(2, 0, 1, 3)
    )


def _pack_w_ktiles(WT):
    """WT [K, M] (contraction-major) -> [MT, P, KT, P] where
    out[mt, p, kt, n] = WT[kt*128+p, mt*128+n] — per-(mt) slab is
    partition-major with [KT, 128] contiguous per partition."""
    K_, M_ = WT.shape
    return np.ascontiguousarray(
        WT.reshape(K_ // 128, 128, M_ // 128, 128).transpose(2, 1, 0, 3)
    )


def _prep_inputs(x, W1, W3, W2, gate_w, A1, B1, A3, B3, A2, B2):
    """Host-side packing: transposes + casts, shared across cores."""
    import ml_dtypes

    bf16 = ml_dtypes.bfloat16
    f32 = np.float32

    xT = np.ascontiguousarray(np.asarray(x, f32).T)            # [D, N]
    dff = W1.shape[0]
    shared = {
        "gate": _sw_d(np.ascontiguousarray(np.asarray(gate_w, f32).T)).astype(bf16),
        "w1t": _pack_w_ktiles(np.asarray(W1, f32).T.astype(bf16)),
        "w3t": _pack_w_ktiles(np.asarray(W3, f32).T.astype(bf16)),
        "w2t": _pack_w_ktiles(np.asarray(W2, f32).T.astype(bf16)),
        "amix": _pack_amix(np.asarray(A1, f32), np.asarray(A3, f32)).astype(bf16),
        "bmix": _pack_bmix(np.asarray(B1, f32), np.asarray(B3, f32)).astype(bf16),
        "a2s": np.ascontiguousarray(
            np.asarray(A2, f32).transpose(2, 0, 1).reshape(dff // 128, 128, -1)
        ).astype(bf16),
        "b2s": _pack_b2(np.asarray(B2, f32)).astype(bf16),
    }
    shared["ident_in"] = np.eye(128, dtype=f32)
    ntok = xT.shape[1] // NCORES
    in_maps = []
    for c in range(NCORES):
        sl = np.ascontiguousarray(xT[:, c * ntok:(c + 1) * ntok])
        m = dict(shared)
        m["x_bf"] = _sw_d(sl.astype(bf16))
        in_maps.append(m)
    return in_maps


def _ensure_compiled():
    if "exec" not in _CACHE:
        nc = build_bass()
        _CACHE["nc"] = nc
        _CACHE["exec"] = _make_exec(nc)
    return _CACHE["exec"]


def _make_exec(nc):
    """Build a jitted 8-core shard_map executor for a Bass program.

    Mirrors concourse.bass2jax.run_bass_via_pjrt, but caches the jitted
    callable and keeps real inputs un-donated so device buffers can be
    reused across calls (for timing)."""
    import jax
    import concourse.mybir as mybir
    from concourse import bass2jax
    from jax.experimental.shard_map import shard_map
    from jax.sharding import Mesh, PartitionSpec

    bass2jax.install_neuronx_cc_hook()

    partition_name = (
        nc.partition_id_tensor.name if nc.partition_id_tensor else None
    )
    in_names, out_names, out_avals, zero_outs = [], [], [], []
    for alloc in nc.m.functions[0].allocations:
        if not isinstance(alloc, mybir.MemoryLocationSet):
            continue
        name = alloc.memorylocations[0].name
        if alloc.kind == "ExternalInput":
            if name != partition_name:
                in_names.append(name)
        elif alloc.kind == "ExternalOutput":
            np_dtype = mybir.dt.np(alloc.dtype)
            out_names.append(name)
            out_avals.append(
                jax.core.ShapedArray(tuple(alloc.tensor_shape), np_dtype)
            )
            zero_outs.append(np.zeros(tuple(alloc.tensor_shape), np_dtype))

    n_params = len(in_names)
    n_outs = len(out_names)
    all_names = in_names + out_names
    if partition_name is not None:
        all_names = all_names + [partition_name]

    def _body(*args):
        operands = list(args)
        if partition_name is not None:
            operands.append(bass2jax.partition_id_tensor())
        outs = bass2jax._bass_exec_p.bind(
            *operands,
            out_avals=tuple(out_avals),
            in_names=tuple(all_names),
            out_names=tuple(out_names),
            lowering_input_output_aliases=(),
            sim_require_finite=True,
            sim_require_nnan=True,
            nc=nc,
        )
        return tuple(outs)

    devices = jax.devices()[:NCORES]
    mesh = Mesh(np.asarray(devices), ("core",))
    in_specs = (PartitionSpec("core"),) * (n_params + n_outs)
    out_specs = (PartitionSpec("core"),) * n_outs
    donate = tuple(range(n_params, n_params + n_outs))
    sharded = jax.jit(
        shard_map(
            _body, mesh=mesh, in_specs=in_specs, out_specs=out_specs,
            check_rep=False,
        ),
        donate_argnums=donate,
        keep_unused=True,
    )
    ctx = {
        "fn": sharded,
        "body": _body,
        "n_operands": n_params + n_outs,
        "in_names": in_names,
        "out_names": out_names,
        "zero_outs": zero_outs,
        "mesh": mesh,
    }
    return ctx


def _concat_inputs(in_maps, in_names):
    return [
        np.concatenate([in_maps[c][nm] for c in range(NCORES)], axis=0)
        for nm in in_names
    ]


def _run(ctx, concat_in):
    zeros = [
        np.zeros((NCORES * z.shape[0], *z.shape[1:]), z.dtype)
        for z in ctx["zero_outs"]
    ]
    return ctx["fn"](*concat_in, *zeros)


def kernel(x, W1, W3, W2, gate_w, A1, B1, A3, B3, A2, B2):
    ctx = _ensure_compiled()
    in_maps = _prep_inputs(x, W1, W3, W2, gate_w, A1, B1, A3, B3, A2, B2)
    concat_in = _concat_inputs(in_maps, ctx["in_names"])
    out_arrs = _run(ctx, concat_in)
    ntok = N // NCORES
    res = np.asarray(out_arrs[ctx["out_names"].index("out_t")])
    res = res.reshape(NCORES, D, ntok)
    out = np.empty((N, D), np.float32)
    for c in range(NCORES):
        out[c * ntok:(c + 1) * ntok, :] = res[c].T
    return out


def time_device(inputs, iters=3, ctx=None):
    """Upload all operands once (no donation), then wall-time jitted runs."""
    import time as _time

    import jax
    from jax.experimental.shard_map import shard_map
    from jax.sharding import NamedSharding, PartitionSpec, Mesh

    if ctx is None:
        ctx = _ensure_compiled()
    if "fn_nodonate" not in ctx:
        ctx["fn_nodonate"] = jax.jit(
            shard_map(
                ctx["body"], mesh=ctx["mesh"],
                in_specs=(PartitionSpec("core"),) * ctx["n_operands"],
                out_specs=(PartitionSpec("core"),) * len(ctx["out_names"]),
                check_rep=False,
            ),
            keep_unused=True,
        )
    fn = ctx["fn_nodonate"]
    in_maps = _prep_inputs(**inputs)
    concat_in = _concat_inputs(in_maps, ctx["in_names"])
    zeros = [
        np.zeros((NCORES * z.shape[0], *z.shape[1:]), z.dtype)
        for z in ctx["zero_outs"]
    ]
    sh = NamedSharding(ctx["mesh"], PartitionSpec("core"))
    dev = [jax.device_put(a, sh) for a in (concat_in + zeros)]
    jax.block_until_ready(fn(*dev))  # warmup/compile
    times = []
    for _ in range(iters):
        t0 = _time.perf_counter()
        jax.block_until_ready(fn(*dev))
        times.append(_time.perf_counter() - t0)
    return min(times)
